# revision 1
# baseline (speedup 1.0000x reference)
"""Trainium2 Bass kernel for a 2-layer GAT occupancy predictor (B=1).

Reference math:
  pts = concat(pos, pos_non_manifold) -> [K=6000, 3]
  mask[i,j] = ||pts_i - pts_j||^2 < 0.05^2          (dense radius graph)
  layer l:  h = x @ Wl                              [K, 4*64]
            e[i,j,h] = leaky02(ed[i,h] + es[j,h])   es/ed = <h, a_src/dst>
            alpha = softmax_j(e masked)
            x' = relu(alpha @ h + b)
  logits = (x2 @ fc_w + fc_b)[M:] reshaped to [1, 2, 3000]

Distribution (8 NeuronCores): nodes are Morton-sorted on the host so the
radius graph becomes block-local; core c owns destination rows
[768c, 768(c+1)) of the sorted, padded 6144-node graph.  The radius mask is
>99.9% empty, so each core only processes the j-tiles (128 sources) that can
reach its destination block: the host computes each core's active-tile list
(conservative epsilon-superset of the exact mask) and pads every list to a
common NSLOT, so all cores run the same program on different slot data.

Layer 1 is fully static: the host passes the gathered slot points.  Between
layers one AllGather shares the transposed features; layer 2 fetches each
slot's x^T tile from the gathered buffer with indirect DMA driven by a
host-computed index table.

Engine mapping per slot:
  PE   : h (with es riding along as 4 extra host-folded weight columns),
         d2 = |p_i - p_j|^2 as one K=5 matmul ([p; sq; 1] x [-2p; 1; sq]),
         the alpha @ h aggregation as bf16 hi+lo pairs into shared PSUM
         banks, denominators as N=1 ones-column matmuls.
  DVE  : scores e = ed+es (bf16 4x mode), T = 0.2e, leaky = max merged over
         heads (2x mode), part of the mask-adds, h hi/lo split.
  ACT  : one head's leaky via Prelu, exp over all 4 heads in one op,
         PSUM->SBUF copies.
  Pool : the other mask-adds, partition-broadcast of ed.
Mask offsets (-30/0 bf16) stream to DRAM in layer 1 and back in layer 2.
Padded nodes sit at (-1,-1,-1): finite features, outside every real radius.
"""

import sys

sys.path.insert(0, "/opt/trn_rl_repo")

from contextlib import ExitStack

import ml_dtypes
import numpy as np

import concourse.bacc as bacc
import concourse.bass as bass
import concourse.mybir as mybir
import concourse.tile as tile
from concourse.bass_utils import run_bass_kernel_spmd

F32 = mybir.dt.float32
BF16 = mybir.dt.bfloat16
I32 = mybir.dt.int32
AF = mybir.ActivationFunctionType
OP = mybir.AluOpType
AX = mybir.AxisListType

N_CORES = 8
N = 3000
M = 3000
K = N + M          # real nodes
KP = 6144          # padded nodes (48 source tiles of 128)
NT = KP // 128     # 48
IC = KP // N_CORES # 768 destinations per core
ICT = IC // 128    # 6 destination chunks per core
H = 4              # heads
C = 64             # channels per head
HC = H * C         # 256
HCE = HC + H       # h columns + es columns
CP1 = C + 1        # head channels + ones column
R2 = float(np.float32(0.05) * np.float32(0.05))
PAD_COORD = -1.0
MASK_EPS = 1e-5    # host activity-test margin (superset of device mask)
MNEG = -30.0       # masked-score offset: exp(-30+L) ~ 1e-12
PAD_TILE = NT - 1  # tile of all-padding nodes, used for unused slots

# engine for the mask-add per (layer, head): Pool offloads DVE
MN_ENGINE = {(1, 0): "pool", (1, 1): "pool", (1, 2): "dve", (1, 3): "dve",
             (2, 0): "pool", (2, 1): "pool", (2, 2): "dve", (2, 3): "dve"}


def build(nslot, n_cores=N_CORES, fake_ag=False, use_prelu=True):
    nc = bacc.Bacc("TRN2", target_bir_lowering=False, debug=False,
                   num_devices=n_cores)
    NS = nslot

    # ---- kernel I/O (identical program on every core) ----
    # pts_sel5 rows: [p(3); sq; ones] for the slot sources
    # pts_own5 rows: [-2p(3); ones; sq] for the own destination columns
    pts_sel5_d = nc.dram_tensor("pts_sel5", [5, NS * 128], F32,
                                kind="ExternalInput")
    pts_own5_d = nc.dram_tensor("pts_own5", [5, IC], F32,
                                kind="ExternalInput")
    pts_own3_d = nc.dram_tensor("pts_own3", [3, IC], F32,
                                kind="ExternalInput")
    agidx_d = nc.dram_tensor("agidx", [128, NS, 2], I32, kind="ExternalInput")
    # w1p/w2p: [W | W @ a_src_blockdiag] so es rides along with h
    w1p_d = nc.dram_tensor("w1p", [3, HCE], F32, kind="ExternalInput")
    w2p_d = nc.dram_tensor("w2p", [HC, HCE], F32, kind="ExternalInput")
    adm1_d = nc.dram_tensor("adm1", [HC, H], BF16, kind="ExternalInput")
    adm2_d = nc.dram_tensor("adm2", [HC, H], BF16, kind="ExternalInput")
    bias1_d = nc.dram_tensor("bias1", [128, HC], F32, kind="ExternalInput")
    bias2_d = nc.dram_tensor("bias2", [128, HC], F32, kind="ExternalInput")
    fcw_d = nc.dram_tensor("fcw", [128, 2 * HC], F32, kind="ExternalInput")
    fcb_d = nc.dram_tensor("fcb", [128, 2], F32, kind="ExternalInput")
    ident_d = nc.dram_tensor("ident", [128, 128], F32, kind="ExternalInput")

    out_d = nc.dram_tensor("out", [IC, 2], F32, kind="ExternalOutput")

    with tile.TileContext(nc) as tc, ExitStack() as st:
        dram = st.enter_context(tc.tile_pool(name="dram", bufs=1,
                                             space="DRAM"))
        xt_bounce = dram.tile([HC, IC], F32)
        ag_out = dram.tile([n_cores * HC, IC], F32,
                           addr_space=("Local" if fake_ag else "Shared"))
        mn_dram = dram.tile([NS, 128, IC], BF16)   # per-slot mask offsets

        const = st.enter_context(tc.tile_pool(name="const", bufs=1))
        pts_sel5_sb = const.tile([5, NS * 128], F32)
        pts_own5_sb = const.tile([5, IC], F32)
        pts_own3_sb = const.tile([3, IC], F32)
        agidx_sb = const.tile([128, NS, 2], I32)
        w1p_sb = const.tile([3, HCE], F32)
        w2p_sb = const.tile([128, 2, HCE], F32)
        adm1_sb = const.tile([128, 2, H], BF16)
        adm2_sb = const.tile([128, 2, H], BF16)
        bias1_sb = const.tile([128, HC], F32)
        bias2_sb = const.tile([128, HC], F32)
        fcw_sb = const.tile([128, 2 * HC], F32)
        fcb_sb = const.tile([128, 2], F32)
        ident_sb = const.tile([128, 128], F32)

        nc.sync.dma_start(out=pts_sel5_sb[:, :], in_=pts_sel5_d[:, :])
        nc.sync.dma_start(out=pts_own5_sb[:, :], in_=pts_own5_d[:, :])
        nc.sync.dma_start(out=pts_own3_sb[:, :], in_=pts_own3_d[:, :])
        nc.sync.dma_start(out=agidx_sb[:, :, :], in_=agidx_d[:, :, :])
        nc.sync.dma_start(out=w1p_sb[:, :], in_=w1p_d[:, :])
        nc.sync.dma_start(out=w2p_sb[:, :, :],
                          in_=w2p_d.rearrange("(s p) c -> p s c", p=128))
        nc.sync.dma_start(out=adm1_sb[:, :, :],
                          in_=adm1_d.rearrange("(s p) h -> p s h", p=128))
        nc.sync.dma_start(out=adm2_sb[:, :, :],
                          in_=adm2_d.rearrange("(s p) h -> p s h", p=128))
        nc.sync.dma_start(out=bias1_sb[:, :], in_=bias1_d[:, :])
        nc.sync.dma_start(out=bias2_sb[:, :], in_=bias2_d[:, :])
        nc.sync.dma_start(out=fcw_sb[:, :], in_=fcw_d[:, :])
        nc.sync.dma_start(out=fcb_sb[:, :], in_=fcb_d[:, :])
        nc.sync.dma_start(out=ident_sb[:, :], in_=ident_d[:, :])

        big = st.enter_context(tc.tile_pool(name="big", bufs=1))
        hp_hi = big.tile([128, NS, H, CP1], BF16)
        hp_lo = big.tile([128, NS, H, CP1], BF16)
        es4 = big.tile([128, NS, H], F32)
        ed_b = big.tile([128, H, IC], BF16)
        x_sb = big.tile([128, ICT, HC], F32)
        xt_own = big.tile([128, 2, IC], F32)
        edt_sb = big.tile([H, IC], BF16)
        edt_rows = big.tile([1, H, IC], BF16)
        logit_sb = big.tile([128, ICT, 2], F32)

        nc.vector.memset(hp_hi[:, :, :, C:CP1], 1.0)
        nc.vector.memset(hp_lo[:, :, :, C:CP1], 0.0)

        ag_flat = ag_out.rearrange("r (b c) -> (r b) c", c=128)

        # ================= the two GAT layers =================
        for layer in (1, 2):
            adm_sb = adm1_sb if layer == 1 else adm2_sb
            bias_sb = bias1_sb if layer == 1 else bias2_sb

            # ---- own-column side: hT(own), edT, ED broadcasts ----
            with tc.tile_pool(name=f"prep{layer}", bufs=2) as prep, \
                 tc.tile_pool(name=f"prep_ps{layer}", bufs=1,
                              space="PSUM") as prep_ps:
                ht_own = prep.tile([128, 2, IC], BF16, tag="ht", bufs=1)
                for oc in range(2):
                    ht_ps = prep_ps.tile([128, IC], F32, tag="ht_ps", bufs=1,
                                         name=f"ht_ps_{layer}_{oc}")
                    if layer == 1:
                        for lo, sz in ((0, 512), (512, 256)):
                            sl = slice(lo, lo + sz)
                            nc.tensor.matmul(
                                ht_ps[:, sl],
                                w1p_sb[:, oc * 128:(oc + 1) * 128],
                                pts_own3_sb[:, sl], start=True, stop=True)
                    else:
                        for s in range(2):
                            for lo, sz in ((0, 512), (512, 256)):
                                sl = slice(lo, lo + sz)
                                nc.tensor.matmul(
                                    ht_ps[:, sl],
                                    w2p_sb[:, s, oc * 128:(oc + 1) * 128],
                                    xt_own[:, s, sl],
                                    start=(s == 0), stop=(s == 1))
                    nc.scalar.copy(ht_own[:, oc, :], ht_ps[:, :])

                edt_ps = prep_ps.tile([H, IC], F32, tag="edt", bufs=1)
                for s in range(2):
                    for lo, sz in ((0, 512), (512, 256)):
                        sl = slice(lo, lo + sz)
                        nc.tensor.matmul(edt_ps[:, sl], adm_sb[:, s, :],
                                         ht_own[:, s, sl],
                                         start=(s == 0), stop=(s == 1))
                nc.scalar.copy(edt_sb[:, :], edt_ps[:, :])
                for h in range(H):
                    nc.sync.dma_start(out=edt_rows[0:1, h, :],
                                      in_=edt_sb[h:h + 1, :])
                for h in range(H):
                    nc.gpsimd.partition_broadcast(ed_b[:, h, :],
                                                  edt_rows[0:1, h, :])

            # ---- slot loop: h+es, d2 mask, scores, aggregation ----
            with tc.tile_pool(name=f"agg_ps{layer}", bufs=1,
                              space="PSUM") as agg_ps:
                aggp = [agg_ps.tile([128, 2, H, C], F32, tag=f"agg{p}",
                                    name=f"agg_{layer}_{p}")
                        for p in range(ICT // 2)]
                den_ps = agg_ps.tile([128, ICT, H], F32, tag="den",
                                     name=f"den_{layer}")
                with tc.tile_pool(name=f"jl{layer}", bufs=3) as jl, \
                     tc.tile_pool(name=f"h_ps{layer}", bufs=2,
                                  space="PSUM") as h_psp:
                    for s in range(NS):
                        # --- h + es for this slot's 128 sources ---
                        h_ps = h_psp.tile([128, HCE], F32, tag="h",
                                          name=f"h_ps_{layer}_{s}")
                        if layer == 1:
                            nc.tensor.matmul(
                                h_ps[:, :],
                                pts_sel5_sb[0:3, s * 128:(s + 1) * 128],
                                w1p_sb[:, :], start=True, stop=True)
                        else:
                            for half in range(2):
                                xtg = jl.tile([128, 128], F32,
                                              tag=f"xtg{half}",
                                              name=f"xtg_{layer}_{s}_{half}")
                                nc.gpsimd.indirect_dma_start(
                                    out=xtg[:, :], out_offset=None,
                                    in_=ag_flat,
                                    in_offset=bass.IndirectOffsetOnAxis(
                                        ap=agidx_sb[:, s, half:half + 1],
                                        axis=0))
                                nc.tensor.matmul(
                                    h_ps[:, :], xtg[:, :],
                                    w2p_sb[:, half, :],
                                    start=(half == 0), stop=(half == 1))
                        nc.vector.tensor_scalar_add(es4[:, s, :],
                                                    h_ps[:, HC:HCE], 0.0)
                        # h -> bf16 hi + lo with ones/zeros column
                        nc.scalar.copy(
                            hp_hi[:, s, :, 0:C],
                            h_ps[:, 0:HC].rearrange("p (h c) -> p h c", h=H))
                        nc.vector.tensor_tensor(
                            hp_lo[:, s, :, 0:C],
                            h_ps[:, 0:HC].rearrange("p (h c) -> p h c", h=H),
                            hp_hi[:, s, :, 0:C], OP.subtract)

                        # --- mask offsets mn (layer 1: d2 on PE; 2: DRAM) ---
                        mn = jl.tile([128, IC], BF16, tag="mn",
                                     name=f"mn_{layer}_{s}")
                        if layer == 1:
                            for lo, sz in ((0, 512), (512, 256)):
                                sl = slice(lo, lo + sz)
                                g_ps = h_psp.tile([128, sz], F32,
                                                  tag=f"g{lo}", bufs=1,
                                                  name=f"g_{s}_{lo}")
                                nc.tensor.matmul(
                                    g_ps[:, :],
                                    pts_sel5_sb[:, s * 128:(s + 1) * 128],
                                    pts_own5_sb[:, sl],
                                    start=True, stop=True)
                                nc.vector.tensor_scalar(
                                    mn[:, sl], g_ps[:, :], R2, MNEG,
                                    OP.is_ge, OP.mult)
                            nc.sync.dma_start(out=mn_dram[s, :, :],
                                              in_=mn[:, :])
                        else:
                            nc.sync.dma_start(out=mn[:, :],
                                              in_=mn_dram[s, :, :])

                        # --- scores: L = leaky(ed+es) + mn ; A = exp(L) ---
                        L4 = jl.tile([128, H, IC], BF16, tag="L4",
                                     name=f"L4_{layer}_{s}")
                        T4 = jl.tile([128, 3, IC], BF16, tag="T4",
                                     name=f"T4_{layer}_{s}")
                        if use_prelu:
                            nc.scalar.activation(L4[:, 0, :], ed_b[:, 0, :],
                                                 AF.Prelu,
                                                 bias=es4[:, s, 0:1],
                                                 scale=1.0, alpha=0.2)
                        else:
                            T0 = jl.tile([128, IC], BF16, tag="T0",
                                         name=f"T0_{layer}_{s}")
                            nc.vector.tensor_scalar(
                                L4[:, 0, :], ed_b[:, 0, :],
                                es4[:, s, 0:1], None, OP.add)
                            nc.vector.tensor_scalar(
                                T0[:, :], L4[:, 0, :], 0.2, None, OP.mult)
                            nc.vector.tensor_tensor(
                                L4[:, 0, :], L4[:, 0, :], T0[:, :], OP.max)
                        for h in range(1, H):
                            nc.vector.tensor_scalar(
                                L4[:, h, :], ed_b[:, h, :],
                                es4[:, s, h:h + 1], None, OP.add)
                            nc.vector.tensor_scalar(
                                T4[:, h - 1, :], L4[:, h, :], 0.2, None,
                                OP.mult)
                        nc.vector.tensor_tensor(L4[:, 1:4, :], L4[:, 1:4, :],
                                                T4[:, :, :], OP.max)
                        for h in range(H):
                            eng = (nc.gpsimd
                                   if MN_ENGINE[(layer, h)] == "pool"
                                   else nc.vector)
                            eng.tensor_tensor(L4[:, h, :], L4[:, h, :],
                                              mn[:, :], OP.add)
                        A4 = jl.tile([128, H, IC], BF16, tag="A4",
                                     name=f"A4_{layer}_{s}")
                        nc.scalar.activation(A4[:, :, :], L4[:, :, :], AF.Exp)

                        # --- aggregation: hi+lo into one psum group per
                        # bank; two ic-chunks share each bank; denominators
                        # (ones column) accumulate in their own bank ---
                        for h in range(H):
                            for ic in range(ICT):
                                out_ap = aggp[ic // 2][:, ic % 2, h, :]
                                first = (s == 0 and h == 0 and ic % 2 == 0)
                                last = (s == NS - 1 and h == H - 1
                                        and ic % 2 == 1)
                                nc.tensor.matmul(
                                    out_ap,
                                    A4[:, h, ic * 128:(ic + 1) * 128],
                                    hp_hi[:, s, h, 0:C],
                                    start=first, stop=False)
                                nc.tensor.matmul(
                                    out_ap,
                                    A4[:, h, ic * 128:(ic + 1) * 128],
                                    hp_lo[:, s, h, 0:C],
                                    start=False, stop=last)
                                nc.tensor.matmul(
                                    den_ps[:, ic, h:h + 1],
                                    A4[:, h, ic * 128:(ic + 1) * 128],
                                    hp_hi[:, s, h, C:CP1],
                                    start=(s == 0 and h == 0 and ic == 0),
                                    stop=(s == NS - 1 and h == H - 1
                                          and ic == ICT - 1))

                # ---- finalize x = relu(num/den + b); AG or fc ----
                with tc.tile_pool(name=f"fin{layer}", bufs=2) as fin, \
                     tc.tile_pool(name=f"fin_ps{layer}", bufs=2,
                                  space="PSUM") as fin_ps:
                    for ic in range(ICT):
                        rec = fin.tile([128, H], F32, tag="rec",
                                       name=f"rec_{layer}_{ic}")
                        nc.vector.reciprocal(rec[:, :], den_ps[:, ic, :])
                        for h in range(H):
                            nc.vector.scalar_tensor_tensor(
                                x_sb[:, ic, h * C:(h + 1) * C],
                                aggp[ic // 2][:, ic % 2, h, :],
                                rec[:, h:h + 1],
                                bias_sb[:, h * C:(h + 1) * C],
                                OP.mult, OP.add)
                    nc.vector.tensor_scalar(x_sb[:, :, :], x_sb[:, :, :],
                                            0.0, None, OP.max)

                    if layer == 1:
                        for ic in range(ICT):
                            for oc in range(2):
                                t_ps = fin_ps.tile([128, 128], F32,
                                                   tag="t_ps",
                                                   name=f"t_ps_{ic}_{oc}")
                                nc.tensor.transpose(
                                    t_ps[:, :],
                                    x_sb[:, ic, oc * 128:(oc + 1) * 128],
                                    ident_sb[:, :])
                                nc.scalar.copy(
                                    xt_own[:, oc, ic * 128:(ic + 1) * 128],
                                    t_ps[:, :])
                        nc.sync.dma_start(
                            out=xt_bounce.rearrange("(s p) i -> p s i",
                                                    p=128),
                            in_=xt_own[:, :, :])
                        if fake_ag:
                            for r in range(n_cores):
                                nc.sync.dma_start(
                                    out=ag_out[r * HC:(r + 1) * HC, :],
                                    in_=xt_bounce[:, :])
                        else:
                            nc.gpsimd.collective_compute(
                                "AllGather", OP.bypass,
                                replica_groups=[list(range(n_cores))],
                                ins=[xt_bounce.opt()],
                                outs=[ag_out.opt()])
                    else:
                        for ic in range(ICT):
                            for o in range(2):
                                prod = fin.tile([128, HC], F32, tag="prod",
                                                name=f"prod_{ic}_{o}")
                                nc.vector.tensor_tensor(
                                    prod[:, :], x_sb[:, ic, :],
                                    fcw_sb[:, o * HC:(o + 1) * HC], OP.mult)
                                red = fin.tile([128, 1], F32, tag="red",
                                               name=f"red_{ic}_{o}")
                                nc.vector.tensor_reduce(
                                    red[:, :], prod[:, :], AX.X, OP.add)
                                nc.vector.tensor_scalar_add(
                                    logit_sb[:, ic, o:o + 1], red[:, :],
                                    fcb_sb[:, o:o + 1])
                        nc.sync.dma_start(
                            out=out_d.rearrange("(q p) o -> p q o", p=128),
                            in_=logit_sb[:, :, :])

    nc.compile()
    return nc


_BUILD_CACHE = {}


def _get_nc(nslot, use_prelu=True):
    key = (nslot, use_prelu)
    if key not in _BUILD_CACHE:
        _BUILD_CACHE[key] = build(nslot, use_prelu=use_prelu)
    return _BUILD_CACHE[key]


def _morton(p, bits=10):
    q = np.clip((p * (1 << bits)).astype(np.int64), 0, (1 << bits) - 1)
    code = np.zeros(len(p), np.int64)
    for b in range(bits):
        for dim in range(3):
            code |= ((q[:, dim] >> b) & 1) << (3 * b + dim)
    return code


def _plan(pts):
    """Sort nodes spatially, find each core's active source tiles."""
    order = np.argsort(_morton(pts), kind="stable")
    p_sorted = np.full((KP, 3), PAD_COORD, np.float32)
    p_sorted[:K] = pts[order]

    sq = (p_sorted ** 2).sum(-1, dtype=np.float32)
    G = p_sorted @ p_sorted.T
    d2 = sq[None, :] + sq[:, None] - 2.0 * G
    near = d2 < (R2 + MASK_EPS)          # [j, i], conservative superset

    jmaps = []
    for c in range(N_CORES):
        cols = near[:, c * IC:(c + 1) * IC]
        act = cols.reshape(NT, 128, IC).any(axis=(1, 2))
        jmaps.append(np.flatnonzero(act))
    nslot = max(len(j) for j in jmaps)
    jmaps = [np.concatenate([j, np.full(nslot - len(j), PAD_TILE, j.dtype)])
             for j in jmaps]
    return order, p_sorted, jmaps, nslot


def _prep_inputs(pos, pos_non_manifold, W1, a_src1, a_dst1, b1,
                 W2, a_src2, a_dst2, b2, fc_w, fc_b):
    bf16 = ml_dtypes.bfloat16
    pts = np.concatenate([np.asarray(pos, np.float32),
                          np.asarray(pos_non_manifold, np.float32)],
                         axis=2)[0].T  # [K, 3]
    order, p_sorted, jmaps, nslot = _plan(pts)
    sq_sorted = (p_sorted ** 2).sum(-1, dtype=np.float32).astype(np.float32)

    def bcast128(v):
        v = np.asarray(v, np.float32).reshape(-1)
        return np.ascontiguousarray(
            np.broadcast_to(v[None, :], (128, v.size)))

    def blockdiag(a):  # [H, C] -> [HC, H] fp32
        m = np.zeros((HC, H), dtype=np.float32)
        for h in range(H):
            m[h * C:(h + 1) * C, h] = np.asarray(a, np.float32)[h]
        return m

    W1f = np.asarray(W1, np.float32)
    W2f = np.asarray(W2, np.float32)
    w1p = np.concatenate([W1f, W1f @ blockdiag(a_src1)], axis=1)
    w2p = np.concatenate([W2f, W2f @ blockdiag(a_src2)], axis=1)

    shared = {
        "w1p": np.ascontiguousarray(w1p.astype(np.float32)),
        "w2p": np.ascontiguousarray(w2p.astype(np.float32)),
        "adm1": blockdiag(a_dst1).astype(bf16),
        "adm2": blockdiag(a_dst2).astype(bf16),
        "bias1": bcast128(b1),
        "bias2": bcast128(b2),
        "fcw": bcast128(np.asarray(fc_w, np.float32).T),
        "fcb": bcast128(fc_b),
        "ident": np.eye(128, dtype=np.float32),
    }
    in_maps = []
    for c in range(N_CORES):
        jm = jmaps[c]
        sel = (jm[:, None] * 128 + np.arange(128)[None, :]).reshape(-1)
        psel = p_sorted[sel]                      # [nslot*128, 3]
        pown = p_sorted[c * IC:(c + 1) * IC]
        sel5 = np.concatenate(
            [psel.T, sq_sorted[sel][None, :],
             np.ones((1, len(sel)), np.float32)], axis=0)
        own5 = np.concatenate(
            [-2.0 * pown.T, np.ones((1, IC), np.float32),
             (pown ** 2).sum(-1, dtype=np.float32)[None, :]], axis=0)
        r = jm // ICT
        lq = jm % ICT
        agidx = np.zeros((128, nslot, 2), np.int32)
        p_ar = np.arange(128)
        for si in range(nslot):
            for half in range(2):
                rows = r[si] * HC + half * 128 + p_ar
                agidx[:, si, half] = rows * ICT + lq[si]
        m = dict(shared)
        m["pts_sel5"] = np.ascontiguousarray(sel5.astype(np.float32))
        m["pts_own5"] = np.ascontiguousarray(own5.astype(np.float32))
        m["pts_own3"] = np.ascontiguousarray(pown.T)
        m["agidx"] = agidx
        in_maps.append(m)
    return in_maps, order, nslot


def kernel(pos, pos_non_manifold, W1, a_src1, a_dst1, b1,
           W2, a_src2, a_dst2, b2, fc_w, fc_b, _trace=False,
           _use_prelu=True):
    in_maps, order, nslot = _prep_inputs(
        pos, pos_non_manifold, W1, a_src1, a_dst1, b1,
        W2, a_src2, a_dst2, b2, fc_w, fc_b)
    nc = _get_nc(nslot, use_prelu=_use_prelu)
    res = run_bass_kernel_spmd(nc, in_maps, core_ids=list(range(N_CORES)),
                               trace=_trace)
    kernel.last_results = res
    x2s = np.concatenate([res.results[c]["out"] for c in range(N_CORES)],
                         axis=0)  # [KP, 2] in sorted order
    x2 = np.empty((K, 2), np.float32)
    x2[order] = x2s[:K]
    logits = np.ascontiguousarray(x2[M:K]).reshape(1, 2, 3000)
    return logits.astype(np.float32)



# revision 20
# speedup vs baseline: 2.5320x; 2.5320x over previous
"""Trainium2 Bass kernel for a 2-layer GAT occupancy predictor (B=1).

Reference math:
  pts = concat(pos, pos_non_manifold) -> [K=6000, 3]
  mask[i,j] = ||pts_i - pts_j||^2 < 0.05^2          (dense radius graph)
  layer l:  h = x @ Wl                              [K, 4*64]
            e[i,j,h] = leaky02(ed[i,h] + es[j,h])   es/ed = <h, a_src/dst>
            alpha = softmax_j(e masked)
            x' = relu(alpha @ h + b)
  logits = (x2 @ fc_w + fc_b)[M:] reshaped to [1, 2, 3000]

Distribution (8 NeuronCores): nodes are Morton-sorted; core c owns the 768
destinations [768c, 768(c+1)) of the padded 6144-node graph.  Each core's
sources are CUSTOM-PACKED: only the ~900 nodes within radius of its block,
gathered into T=ceil(max_unique/128) tiles of 128 (padded with node 6143),
instead of whole global Morton tiles.  This cuts per-core source tiles from
~28 to ~8 and makes dense-768-dst processing cheap enough to skip chunking.

Everything 16-bit on the hot path (fp16), f32 accumulation in PSUM:
  per slot s (128 sources x 768 dsts x 4 heads):
    PE   : layer1 h = p @ W1 [128,256]; g = (R2-d2 | es-cols) via one K=5
           matmul; transposed aggregation x^T[c,dst] += A.h with [h|ones]
           stationary (denominator rides as the 65th weight column).
    gpsimd: mask mn = (g<0)*-60000 (psum->fp16), layer-2 row gathers
           (indirect DMA), ed/deninv partition broadcasts.
    DVE  : per head ONE fused v_h = (ed_h + es_h) + mn  (scalar_tensor_tensor)
           then ONE batched leaky L = max(0.2v, v) over all heads.
    ACT  : one exp over [128, 4*768], h copies.
  Between layers: x1^T assembled by 4 partition-moving DMAs; h2 = x1 @ W2
  (+es ride-along) computed per-owner, AllGathered as fp16 node-major rows
  [h0|1|h1|1|h2|1|h3|1|es4]; layer 2 fetches each slot's rows with one
  indirect DMA.  Masks bounce through DRAM between layers.
"""

import sys

sys.path.insert(0, "/opt/trn_rl_repo")

from contextlib import ExitStack

import ml_dtypes
import numpy as np

import concourse.bacc as bacc
import concourse.bass as bass
import concourse.mybir as mybir
import concourse.tile as tile
from concourse.bass_utils import run_bass_kernel_spmd

F32 = mybir.dt.float32
F16 = mybir.dt.float16
I32 = mybir.dt.int32
AF = mybir.ActivationFunctionType
OP = mybir.AluOpType
AX = mybir.AxisListType

N_CORES = 8
N = 3000
M = 3000
K = N + M          # real nodes
KP = 6144          # padded nodes
IC = KP // N_CORES # 768 destinations per core
H = 4              # heads
C = 64             # channels per head
HC = H * C         # 256
HCE = HC + H       # 260: h columns + es columns (layer-2 ride-along)
ROWW = H * (C + 1) + H  # 264: AG row [h0|1|h1|1|h2|1|h3|1|es4]
R2 = float(np.float32(0.05) * np.float32(0.05))
PAD_COORD = -1.0
PAD_NODE = KP - 1
MASK_EPS = 1e-5    # host activity-test margin (superset of device mask)
MNEG = -60000.0    # masked-score offset; *0.2 then exp -> 0 in fp16
GA = 384           # d2/mask column chunk (PSUM bank budget)


def build(nslot, n_cores=N_CORES, fake_ag=False, dbg=False):
    nc = bacc.Bacc("TRN2", target_bir_lowering=False, debug=False,
                   num_devices=n_cores)
    T = nslot
    dbg_d = {}
    if dbg:
        for nm, shp, dt in (("dbg_den", [1, H * IC], F32),
                            ("dbg_dinv", [1, H * IC], F32),
                            ("dbg_x1T", [128, 2, IC], F16),
                            ("dbg_edb", [128, H, IC], F16),
                            ("dbg_mn0", [128, IC], F16),
                            ("dbg_A0", [128, H, IC], F16),
                            ("dbg_hsrc", [128, nslot, ROWW], F16)):
            dbg_d[nm] = nc.dram_tensor(nm, shp, dt, kind="ExternalOutput")

    # ---- kernel I/O (identical program on every core) ----
    sel5_d = nc.dram_tensor("sel5", [5, T * 128], F32, kind="ExternalInput")
    # own5ge: cols 0:768 = [2p; -1; R2-sq] (g = R2-d2), cols 768:772 = es1
    own5ge_d = nc.dram_tensor("own5ge", [5, IC + H], F32,
                              kind="ExternalInput")
    own3_d = nc.dram_tensor("own3", [3, IC], F32, kind="ExternalInput")
    agidx_d = nc.dram_tensor("agidx", [128, T], I32, kind="ExternalInput")
    w1p_d = nc.dram_tensor("w1p", [3, HC], F32, kind="ExternalInput")
    w1d_d = nc.dram_tensor("w1d", [3, H], F32, kind="ExternalInput")
    w2p_d = nc.dram_tensor("w2p", [HC, HCE], F16, kind="ExternalInput")
    admw2_d = nc.dram_tensor("admw2", [HC, H], F16, kind="ExternalInput")
    b1t_d = nc.dram_tensor("b1t", [C, H], F32, kind="ExternalInput")
    b2t_d = nc.dram_tensor("b2t", [C, H], F32, kind="ExternalInput")
    fcw_d = nc.dram_tensor("fcw", [HC, 2], F16, kind="ExternalInput")
    fcb_d = nc.dram_tensor("fcb", [128, 2], F32, kind="ExternalInput")

    out_d = nc.dram_tensor("out", [IC, 2], F32, kind="ExternalOutput")

    with tile.TileContext(nc) as tc, ExitStack() as st:
        dram = st.enter_context(tc.tile_pool(name="dram", bufs=1,
                                             space="DRAM"))
        mn_dram = dram.tile([T, 128, IC], F16)
        den_dram = dram.tile([2, H * IC], F32)
        hg_dram = dram.tile([IC, ROWW], F16)
        ag_out = dram.tile([KP, ROWW], F16,
                           addr_space=("Local" if fake_ag else "Shared"))

        const = st.enter_context(tc.tile_pool(name="const", bufs=1))
        sel5_sb = const.tile([5, T * 128], F32)
        own5ge_sb = const.tile([5, IC + H], F32)
        own3_sb = const.tile([3, IC], F32)
        agidx_sb = const.tile([128, T], I32)
        w1p_sb = const.tile([3, HC], F32)
        w1d_sb = const.tile([3, H], F32)
        w2p_sb = const.tile([128, 2, HCE], F16)
        admw2_sb = const.tile([128, 2, H], F16)
        b1t_sb = const.tile([C, H], F32)
        b2t_sb = const.tile([C, H], F32)
        fcw_sb = const.tile([128, 2, 2], F16)
        fcb_sb = const.tile([128, 2], F32)

        nc.sync.dma_start(out=sel5_sb[:, :], in_=sel5_d[:, :])
        nc.sync.dma_start(out=own5ge_sb[:, :], in_=own5ge_d[:, :])
        nc.sync.dma_start(out=own3_sb[:, :], in_=own3_d[:, :])
        nc.sync.dma_start(out=agidx_sb[:, :], in_=agidx_d[:, :])
        nc.sync.dma_start(out=w1p_sb[:, :], in_=w1p_d[:, :])
        nc.sync.dma_start(out=w1d_sb[:, :], in_=w1d_d[:, :])
        nc.sync.dma_start(out=w2p_sb[:, :, :],
                          in_=w2p_d.rearrange("(s p) c -> p s c", p=128))
        nc.sync.dma_start(out=admw2_sb[:, :, :],
                          in_=admw2_d.rearrange("(s p) h -> p s h", p=128))
        nc.sync.dma_start(out=b1t_sb[:, :], in_=b1t_d[:, :])
        nc.sync.dma_start(out=b2t_sb[:, :], in_=b2t_d[:, :])
        nc.sync.dma_start(out=fcw_sb[:, :, :],
                          in_=fcw_d.rearrange("(s p) o -> p s o", p=128))
        nc.sync.dma_start(out=fcb_sb[:, :], in_=fcb_d[:, :])

        big = st.enter_context(tc.tile_pool(name="big", bufs=1))
        # layer-1 source features, AG-row layout [h0|1|h1|1|h2|1|h3|1|es4]
        hsrc = big.tile([128, T, ROWW], F16)
        ed_b = big.tile([128, H, IC], F16)
        edt_sb = big.tile([H, IC], F16)
        edt_row = big.tile([1, H, IC], F16)
        x1T = big.tile([128, 2, IC], F16)
        x2T = big.tile([128, 2, IC], F16)
        hg_sb = big.tile([128, IC // 128, ROWW], F16)
        den_sb = big.tile([128, H * IC], F32)
        dinv_sb = big.tile([128, H * IC], F32)
        dinv_row = big.tile([1, H * IC], F32)
        dinv_b = big.tile([128, H, IC], F32)
        logit_sb = big.tile([128, IC // 128, 2], F32)

        h65 = hsrc[:, :, 0:H * (C + 1)].rearrange("p t (h x) -> p t h x", h=H)
        nc.vector.memset(h65[:, :, :, C:C + 1], 1.0)
        g65 = hg_sb[:, :, 0:H * (C + 1)].rearrange("p q (h x) -> p q h x",
                                                   h=H)
        nc.vector.memset(g65[:, :, :, C:C + 1], 1.0)

        for layer in (1, 2):
            # ---- prep: edt rows + partition-broadcast to ed_b ----
            with tc.tile_pool(name=f"prep{layer}", bufs=1,
                              space="PSUM") as prep_ps:
                edt_ps = prep_ps.tile([H, IC], F32, tag="edt")
                for lo, sz in ((0, 512), (512, 256)):
                    sl = slice(lo, lo + sz)
                    if layer == 1:
                        nc.tensor.matmul(edt_ps[:, sl], w1d_sb[:, :],
                                         own3_sb[:, sl],
                                         start=True, stop=True)
                    else:
                        for s2 in range(2):
                            nc.tensor.matmul(edt_ps[:, sl],
                                             admw2_sb[:, s2, :],
                                             x1T[:, s2, sl],
                                             start=(s2 == 0), stop=(s2 == 1))
                nc.scalar.copy(edt_sb[:, :], edt_ps[:, :])
            for h in range(H):
                nc.sync.dma_start(out=edt_row[0:1, h, :],
                                  in_=edt_sb[h:h + 1, :])
            for h in range(H):
                nc.gpsimd.partition_broadcast(ed_b[:, h, :],
                                              edt_row[0:1, h, :])

            # ---- slot loop ----
            with tc.tile_pool(name=f"agg_ps{layer}", bufs=1,
                              space="PSUM") as agg_pool:
                agg_ps = agg_pool.tile([128, H, IC], F32, tag="agg",
                                       name=f"agg_{layer}")
                with tc.tile_pool(name=f"jl{layer}", bufs=2) as jl, \
                     tc.tile_pool(name=f"mnp{layer}", bufs=3) as mnp, \
                     tc.tile_pool(name=f"h_ps{layer}", bufs=1,
                                  space="PSUM") as h_psp:
                    for s in range(T):
                        mn = mnp.tile([128, IC], F16, tag="mn",
                                      name=f"mn_{layer}_{s}")
                        if layer == 1:
                            h_ps = h_psp.tile([128, HC], F32, tag="h",
                                              name=f"h_ps_{s}")
                            nc.tensor.matmul(
                                h_ps[:, :],
                                sel5_sb[0:3, s * 128:(s + 1) * 128],
                                w1p_sb[:, :], start=True, stop=True)
                            nc.scalar.copy(
                                h65[:, s, :, 0:C],
                                h_ps[:, :].rearrange("p (h c) -> p h c",
                                                     h=H))
                            # g = R2-d2 (+es cols on 2nd chunk), 2 chunks
                            g_ps = h_psp.tile([128, GA + H], F32, tag="g",
                                              name=f"g_ps_{s}")
                            nc.tensor.matmul(
                                g_ps[:, 0:GA],
                                sel5_sb[:, s * 128:(s + 1) * 128],
                                own5ge_sb[:, 0:GA], start=True, stop=True)
                            nc.vector.tensor_scalar(
                                mn[:, 0:GA], g_ps[:, 0:GA], 0.0, MNEG,
                                OP.is_lt, OP.mult)
                            nc.tensor.matmul(
                                g_ps[:, :],
                                sel5_sb[:, s * 128:(s + 1) * 128],
                                own5ge_sb[:, GA:IC + H],
                                start=True, stop=True)
                            nc.vector.tensor_scalar(
                                mn[:, GA:IC], g_ps[:, 0:IC - GA], 0.0, MNEG,
                                OP.is_lt, OP.mult)
                            nc.vector.tensor_scalar_add(
                                hsrc[:, s, H * (C + 1):ROWW],
                                g_ps[:, IC - GA:IC - GA + H], 0.0)
                            nc.sync.dma_start(out=mn_dram[s, :, :],
                                              in_=mn[:, :])
                            src = hsrc[:, s, :]
                        else:
                            src = jl.tile([128, ROWW], F16, tag="hg",
                                          name=f"hg_{s}")
                            nc.gpsimd.indirect_dma_start(
                                out=src[:, :], out_offset=None,
                                in_=ag_out[:, :],
                                in_offset=bass.IndirectOffsetOnAxis(
                                    ap=agidx_sb[:, s:s + 1], axis=0))
                            nc.sync.dma_start(out=mn[:, :],
                                              in_=mn_dram[s, :, :])

                        # scores: v = (ed + es) + mn; L = max(.2v, v); A=exp
                        v4 = jl.tile([128, H, IC], F16, tag="v4",
                                     name=f"v4_{layer}_{s}")
                        for h in range(H):
                            nc.vector.scalar_tensor_tensor(
                                v4[:, h, :], ed_b[:, h, :],
                                src[:, H * (C + 1) + h:H * (C + 1) + h + 1],
                                mn[:, :], OP.add, OP.add)
                        L4 = jl.tile([128, H, IC], F16, tag="L4",
                                     name=f"L4_{layer}_{s}")
                        nc.vector.scalar_tensor_tensor(
                            L4[:, :, :], v4[:, :, :], 0.2, v4[:, :, :],
                            OP.mult, OP.max)
                        A4 = jl.tile([128, H, IC], F16, tag="A4",
                                     name=f"A4_{layer}_{s}")
                        nc.scalar.activation(A4[:, :, :], L4[:, :, :], AF.Exp)
                        if dbg and layer == 1 and s == 0:
                            nc.sync.dma_start(out=dbg_d["dbg_mn0"][:, :],
                                              in_=mn[:, :])
                            nc.sync.dma_start(out=dbg_d["dbg_A0"][:, :, :],
                                              in_=A4[:, :, :])

                        # transposed aggregation: [h|ones] stationary.
                        # 256-col (1KB) chunks keep every matmul output
                        # inside one PSUM bank (head stride is 3KB).
                        # start=True clears has_written for the WHOLE bank,
                        # so only the first-issued region of each bank may
                        # set it (those with q ≡ h mod 2); the bank-mate
                        # region's first write then lands in overwrite mode.
                        for h in range(H):
                            for q in range(3):
                                lo = q * 256
                                nc.tensor.matmul(
                                    agg_ps[0:C + 1, h, lo:lo + 256],
                                    src[:, h * (C + 1):(h + 1) * (C + 1)],
                                    A4[:, h, lo:lo + 256],
                                    start=(s == 0 and (q % 2) == (h % 2)),
                                    stop=(s == T - 1))

                # ---- finalize: x^T = relu(num*dinv + b) ----
                bt_sb = b1t_sb if layer == 1 else b2t_sb
                xT = x1T if layer == 1 else x2T
                with tc.tile_pool(name=f"fin{layer}", bufs=1) as fin:
                    # den: PSUM -> SBUF row -> [128,24] for a lane-parallel
                    # exact reciprocal -> back to a partition-0 row
                    nc.scalar.copy(
                        den_sb[C:C + 1, :],
                        agg_ps[C:C + 1, :, :].rearrange("p h d -> p (h d)"))
                    nc.sync.dma_start(out=den_dram[0, :],
                                      in_=den_sb[C:C + 1, :])
                    den_t128 = fin.tile([128, (H * IC) // 128], F32,
                                        tag="den128")
                    nc.sync.dma_start(
                        out=den_t128[:, :],
                        in_=den_dram[0, :].rearrange("(a b) -> a b", a=128))
                    dinv_t128 = fin.tile([128, (H * IC) // 128], F32,
                                         tag="dinv128")
                    nc.vector.reciprocal(dinv_t128[:, :], den_t128[:, :])
                    nc.sync.dma_start(
                        out=den_dram[1, :].rearrange("(a b) -> a b", a=128),
                        in_=dinv_t128[:, :])
                    nc.sync.dma_start(out=dinv_row[0:1, :],
                                      in_=den_dram[1, :])
                    nc.gpsimd.partition_broadcast(
                        dinv_b[0:C, :, :].rearrange("p h d -> p (h d)"),
                        dinv_row[0:1, :])
                    if dbg and layer == 1:
                        nc.sync.dma_start(out=dbg_d["dbg_den"][:, :],
                                          in_=den_sb[C:C + 1, :])
                        nc.sync.dma_start(out=dbg_d["dbg_dinv"][:, :],
                                          in_=dinv_row[0:1, :])
                    xc = fin.tile([C, H, IC], F16, tag="xc")
                    nc.vector.tensor_tensor(
                        xc[:, :, :], agg_ps[0:C, :, :], dinv_b[0:C, :, :],
                        OP.mult)
                    xr = fin.tile([C, H, IC], F16, tag="xr")
                    for h in range(H):
                        nc.vector.tensor_scalar(
                            xr[:, h, :], xc[:, h, :], bt_sb[:, h:h + 1],
                            0.0, OP.add, OP.max)
                    for h in range(H):
                        po = (h % 2) * C
                        nc.sync.dma_start(
                            out=xT[po:po + C, h // 2, :],
                            in_=xr[0:C, h, :])

            if dbg and layer == 1:
                nc.sync.dma_start(out=dbg_d["dbg_x1T"][:, :, :],
                                  in_=x1T[:, :, :])
                nc.sync.dma_start(out=dbg_d["dbg_edb"][:, :, :],
                                  in_=ed_b[:, :, :])
                nc.sync.dma_start(out=dbg_d["dbg_hsrc"][:, :, :],
                                  in_=hsrc[:, :, :])
            if layer == 1:
                # ---- h2 rows (+es) for all own nodes; AllGather ----
                with tc.tile_pool(name="h2", bufs=2, space="PSUM") as h2p:
                    for oc in range(IC // 128):
                        h2_ps = h2p.tile([128, HCE], F32, tag="h2",
                                         name=f"h2_{oc}")
                        for s2 in range(2):
                            nc.tensor.matmul(
                                h2_ps[:, :],
                                x1T[:, s2, oc * 128:(oc + 1) * 128],
                                w2p_sb[:, s2, :],
                                start=(s2 == 0), stop=(s2 == 1))
                        nc.scalar.copy(
                            g65[:, oc, :, 0:C],
                            h2_ps[:, 0:HC].rearrange("p (h c) -> p h c",
                                                     h=H))
                        nc.vector.tensor_scalar_add(
                            hg_sb[:, oc, H * (C + 1):ROWW],
                            h2_ps[:, HC:HCE], 0.0)
                nc.sync.dma_start(
                    out=hg_dram.rearrange("(q p) r -> p q r", p=128),
                    in_=hg_sb[:, :, :])
                if fake_ag:
                    for r in range(n_cores):
                        nc.sync.dma_start(
                            out=ag_out[r * IC:(r + 1) * IC, :],
                            in_=hg_dram[:, :])
                else:
                    nc.gpsimd.collective_compute(
                        "AllGather", OP.bypass,
                        replica_groups=[list(range(n_cores))],
                        ins=[hg_dram.opt()],
                        outs=[ag_out.opt()])
            else:
                # ---- fc head ----
                with tc.tile_pool(name="fc", bufs=1, space="PSUM") as fcp:
                    logit_ps = fcp.tile([128, IC // 128, 2], F32, tag="lg")
                    # all 6 chunks share one PSUM bank: single start=True
                    for oc in range(IC // 128):
                        for s2 in range(2):
                            nc.tensor.matmul(
                                logit_ps[:, oc, :],
                                x2T[:, s2, oc * 128:(oc + 1) * 128],
                                fcw_sb[:, s2, :],
                                start=(oc == 0 and s2 == 0), stop=(s2 == 1))
                    for o in range(2):
                        nc.vector.tensor_scalar_add(
                            logit_sb[:, :, o], logit_ps[:, :, o],
                            fcb_sb[:, o:o + 1])
                nc.sync.dma_start(
                    out=out_d.rearrange("(q p) o -> p q o", p=128),
                    in_=logit_sb[:, :, :])

    nc.compile()
    return nc


_BUILD_CACHE = {}


def _get_nc(nslot):
    if nslot not in _BUILD_CACHE:
        _BUILD_CACHE[nslot] = build(nslot)
    return _BUILD_CACHE[nslot]


def _morton(p, bits=10):
    q = np.clip((p * (1 << bits)).astype(np.int64), 0, (1 << bits) - 1)
    code = np.zeros(len(p), np.int64)
    for b in range(bits):
        for dim in range(3):
            code |= ((q[:, dim] >> b) & 1) << (3 * b + dim)
    return code


def _plan(pts):
    """Sort nodes spatially; pick each core's relevant-source node list."""
    order = np.argsort(_morton(pts), kind="stable")
    p_sorted = np.full((KP, 3), PAD_COORD, np.float32)
    p_sorted[:K] = pts[order]

    sq = (p_sorted ** 2).sum(-1, dtype=np.float32)
    G = p_sorted @ p_sorted.T
    d2 = sq[None, :] + sq[:, None] - 2.0 * G
    near = d2 < (R2 + MASK_EPS)          # [src, dst], conservative superset

    srcs_list = []
    for c in range(N_CORES):
        srcs = np.flatnonzero(near[:, c * IC:(c + 1) * IC].any(axis=1))
        srcs_list.append(srcs)
    T = max(-(-len(s) // 128) for s in srcs_list)
    srcs_list = [np.concatenate(
        [s, np.full(T * 128 - len(s), PAD_NODE, s.dtype)])
        for s in srcs_list]
    return order, p_sorted, srcs_list, T


def _blockdiag(a):  # [H, C] -> [HC, H] fp32
    m = np.zeros((HC, H), dtype=np.float32)
    for h in range(H):
        m[h * C:(h + 1) * C, h] = np.asarray(a, np.float32)[h]
    return m


def _prep_inputs(pos, pos_non_manifold, W1, a_src1, a_dst1, b1,
                 W2, a_src2, a_dst2, b2, fc_w, fc_b):
    f16 = np.float16
    pts = np.concatenate([np.asarray(pos, np.float32),
                          np.asarray(pos_non_manifold, np.float32)],
                         axis=2)[0].T  # [K, 3]
    order, p_sorted, srcs_list, T = _plan(pts)
    sq_sorted = (p_sorted ** 2).sum(-1, dtype=np.float32)

    W1f = np.asarray(W1, np.float32)
    W2f = np.asarray(W2, np.float32)
    w1s = W1f @ _blockdiag(a_src1)            # [3, H]
    w2p = np.concatenate([W2f, W2f @ _blockdiag(a_src2)], axis=1)

    shared = {
        "w1p": np.ascontiguousarray(W1f),
        "w1d": np.ascontiguousarray(W1f @ _blockdiag(a_dst1)),
        "w2p": np.ascontiguousarray(w2p.astype(f16)),
        "admw2": np.ascontiguousarray(
            (W2f @ _blockdiag(a_dst2)).astype(f16)),
        "b1t": np.ascontiguousarray(
            np.asarray(b1, np.float32).reshape(H, C).T),
        "b2t": np.ascontiguousarray(
            np.asarray(b2, np.float32).reshape(H, C).T),
        "fcw": np.ascontiguousarray(np.asarray(fc_w, np.float32).astype(f16)),
        "fcb": np.ascontiguousarray(np.broadcast_to(
            np.asarray(fc_b, np.float32).reshape(1, 2), (128, 2))),
    }
    in_maps = []
    for c in range(N_CORES):
        srcs = srcs_list[c]
        psel = p_sorted[srcs]                     # [T*128, 3]
        pown = p_sorted[c * IC:(c + 1) * IC]
        sel5 = np.concatenate(
            [psel.T, sq_sorted[srcs][None, :],
             np.ones((1, len(srcs)), np.float32)], axis=0)
        own5 = np.concatenate(
            [2.0 * pown.T, -np.ones((1, IC), np.float32),
             (R2 - sq_sorted[c * IC:(c + 1) * IC])[None, :]], axis=0)
        es_cols = np.concatenate(
            [w1s, np.zeros((2, H), np.float32)], axis=0)  # [5, H]
        m = dict(shared)
        m["sel5"] = np.ascontiguousarray(sel5)
        m["own5ge"] = np.ascontiguousarray(
            np.concatenate([own5, es_cols], axis=1))
        m["own3"] = np.ascontiguousarray(pown.T)
        m["agidx"] = np.ascontiguousarray(
            srcs.reshape(T, 128).T.astype(np.int32))
        in_maps.append(m)
    return in_maps, order, T


def kernel(pos, pos_non_manifold, W1, a_src1, a_dst1, b1,
           W2, a_src2, a_dst2, b2, fc_w, fc_b, _trace=False):
    in_maps, order, T = _prep_inputs(
        pos, pos_non_manifold, W1, a_src1, a_dst1, b1,
        W2, a_src2, a_dst2, b2, fc_w, fc_b)
    nc = _get_nc(T)
    res = run_bass_kernel_spmd(nc, in_maps, core_ids=list(range(N_CORES)),
                               trace=_trace)
    kernel.last_results = res
    x2s = np.concatenate([res.results[c]["out"] for c in range(N_CORES)],
                         axis=0)  # [KP, 2] in sorted order
    x2 = np.empty((K, 2), np.float32)
    x2[order] = x2s[:K]
    logits = np.ascontiguousarray(x2[M:K]).reshape(1, 2, 3000)
    return logits.astype(np.float32)


# revision 24
# speedup vs baseline: 2.8366x; 1.1203x over previous
"""Trainium2 Bass kernel for a 2-layer GAT occupancy predictor (B=1).

Reference math:
  pts = concat(pos, pos_non_manifold) -> [K=6000, 3]
  mask[i,j] = ||pts_i - pts_j||^2 < 0.05^2          (dense radius graph)
  layer l:  h = x @ Wl                              [K, 4*64]
            e[i,j,h] = leaky02(ed[i,h] + es[j,h])   es/ed = <h, a_src/dst>
            alpha = softmax_j(e masked)
            x' = relu(alpha @ h + b)
  logits = (x2 @ fc_w + fc_b)[M:] reshaped to [1, 2, 3000]

Distribution (8 NeuronCores): nodes are Morton-sorted; core c owns the 768
destinations [768c, 768(c+1)) of the padded 6144-node graph.  Each core's
sources are CUSTOM-PACKED: only the ~900 nodes within radius of its block,
gathered into T=ceil(max_unique/128) tiles of 128 (padded with node 6143),
instead of whole global Morton tiles.  This cuts per-core source tiles from
~28 to ~8 and makes dense-768-dst processing cheap enough to skip chunking.

Everything 16-bit on the hot path (fp16), f32 accumulation in PSUM:
  per slot s (128 sources x 768 dsts x 4 heads):
    PE   : layer1 h = p @ W1 [128,256]; g = (R2-d2 | es-cols) via one K=5
           matmul; transposed aggregation x^T[c,dst] += A.h with [h|ones]
           stationary (denominator rides as the 65th weight column).
    gpsimd: mask mn = (g<0)*-60000 (psum->fp16), layer-2 row gathers
           (indirect DMA), ed/deninv partition broadcasts.
    DVE  : per head ONE fused v_h = (ed_h + es_h) + mn  (scalar_tensor_tensor)
           then ONE batched leaky L = max(0.2v, v) over all heads.
    ACT  : one exp over [128, 4*768], h copies.
  Between layers: x1^T assembled by 4 partition-moving DMAs; h2 = x1 @ W2
  (+es ride-along) computed per-owner, AllGathered as fp16 node-major rows
  [h0|1|h1|1|h2|1|h3|1|es4]; layer 2 fetches each slot's rows with one
  indirect DMA.  Masks bounce through DRAM between layers.
"""

import sys

sys.path.insert(0, "/opt/trn_rl_repo")

from contextlib import ExitStack

import ml_dtypes
import numpy as np

import concourse.bacc as bacc
import concourse.bass as bass
import concourse.mybir as mybir
import concourse.tile as tile
from concourse.bass_utils import run_bass_kernel_spmd

F32 = mybir.dt.float32
F16 = mybir.dt.float16
I32 = mybir.dt.int32
AF = mybir.ActivationFunctionType
OP = mybir.AluOpType
AX = mybir.AxisListType

N_CORES = 8
N = 3000
M = 3000
K = N + M          # real nodes
KP = 6144          # padded nodes
IC = KP // N_CORES # 768 destinations per core
H = 4              # heads
C = 64             # channels per head
HC = H * C         # 256
HCE = HC + H       # 260: h columns + es columns (layer-2 ride-along)
ROWW = H * (C + 1) + H  # 264: AG row [h0|1|h1|1|h2|1|h3|1|es4]
R2 = float(np.float32(0.05) * np.float32(0.05))
PAD_COORD = -1.0
PAD_NODE = KP - 1
MASK_EPS = 1e-5    # host activity-test margin (superset of device mask)
MNEG = -60000.0    # masked-score offset; *0.2 then exp -> 0 in fp16
GA = 384           # d2/mask column chunk (PSUM bank budget)


def build(nslot, n_cores=N_CORES, fake_ag=False, dbg=False):
    nc = bacc.Bacc("TRN2", target_bir_lowering=False, debug=False,
                   num_devices=n_cores)
    T = nslot
    dbg_d = {}
    if dbg:
        for nm, shp, dt in (("dbg_den", [1, H * IC], F32),
                            ("dbg_dinv", [1, H * IC], F32),
                            ("dbg_x1T", [128, 2, IC], F16),
                            ("dbg_edb", [128, H, IC], F16),
                            ("dbg_mn0", [128, IC], F16),
                            ("dbg_A0", [128, H, IC], F16),
                            ("dbg_hsrc", [128, nslot, ROWW], F16)):
            dbg_d[nm] = nc.dram_tensor(nm, shp, dt, kind="ExternalOutput")

    # ---- kernel I/O (identical program on every core) ----
    sel5_d = nc.dram_tensor("sel5", [5, T * 128], F32, kind="ExternalInput")
    # own5ge: cols 0:768 = [2p; -1; R2-sq] (g = R2-d2), cols 768:772 = es1
    own5ge_d = nc.dram_tensor("own5ge", [5, IC + H], F32,
                              kind="ExternalInput")
    own3_d = nc.dram_tensor("own3", [3, IC], F32, kind="ExternalInput")
    agidx_d = nc.dram_tensor("agidx", [128, T], I32, kind="ExternalInput")
    w1p_d = nc.dram_tensor("w1p", [3, HC], F32, kind="ExternalInput")
    w1d_d = nc.dram_tensor("w1d", [3, H], F32, kind="ExternalInput")
    w2p_d = nc.dram_tensor("w2p", [HC, HCE], F16, kind="ExternalInput")
    admw2_d = nc.dram_tensor("admw2", [HC, H], F16, kind="ExternalInput")
    b1t_d = nc.dram_tensor("b1t", [C, H], F32, kind="ExternalInput")
    b2t_d = nc.dram_tensor("b2t", [C, H], F32, kind="ExternalInput")
    fcw_d = nc.dram_tensor("fcw", [HC, 2], F16, kind="ExternalInput")
    fcb_d = nc.dram_tensor("fcb", [128, 2], F32, kind="ExternalInput")

    out_d = nc.dram_tensor("out", [IC, 2], F32, kind="ExternalOutput")

    with tile.TileContext(nc) as tc, ExitStack() as st:
        dram = st.enter_context(tc.tile_pool(name="dram", bufs=1,
                                             space="DRAM"))
        mn_dram = dram.tile([T, 128, IC], F16)
        den_dram = dram.tile([2, H * IC], F32)
        hg_dram = dram.tile([IC, ROWW], F16)
        ag_out = dram.tile([KP, ROWW], F16,
                           addr_space=("Local" if fake_ag else "Shared"))

        const = st.enter_context(tc.tile_pool(name="const", bufs=1))
        sel5_sb = const.tile([5, T * 128], F32)
        own5ge_sb = const.tile([5, IC + H], F32)
        own3_sb = const.tile([3, IC], F32)
        agidx_sb = const.tile([128, T], I32)
        w1p_sb = const.tile([3, HC], F32)
        w1d_sb = const.tile([3, H], F32)
        w2p_sb = const.tile([128, 2, HCE], F16)
        admw2_sb = const.tile([128, 2, H], F16)
        b1t_sb = const.tile([C, H], F32)
        b2t_sb = const.tile([C, H], F32)
        fcw_sb = const.tile([128, 2, 2], F16)
        fcb_sb = const.tile([128, 2], F32)

        nc.sync.dma_start(out=sel5_sb[:, :], in_=sel5_d[:, :])
        nc.sync.dma_start(out=own5ge_sb[:, :], in_=own5ge_d[:, :])
        nc.sync.dma_start(out=own3_sb[:, :], in_=own3_d[:, :])
        nc.sync.dma_start(out=agidx_sb[:, :], in_=agidx_d[:, :])
        nc.sync.dma_start(out=w1p_sb[:, :], in_=w1p_d[:, :])
        nc.sync.dma_start(out=w1d_sb[:, :], in_=w1d_d[:, :])
        nc.sync.dma_start(out=w2p_sb[:, :, :],
                          in_=w2p_d.rearrange("(s p) c -> p s c", p=128))
        nc.sync.dma_start(out=admw2_sb[:, :, :],
                          in_=admw2_d.rearrange("(s p) h -> p s h", p=128))
        nc.sync.dma_start(out=b1t_sb[:, :], in_=b1t_d[:, :])
        nc.sync.dma_start(out=b2t_sb[:, :], in_=b2t_d[:, :])
        nc.sync.dma_start(out=fcw_sb[:, :, :],
                          in_=fcw_d.rearrange("(s p) o -> p s o", p=128))
        nc.sync.dma_start(out=fcb_sb[:, :], in_=fcb_d[:, :])

        big = st.enter_context(tc.tile_pool(name="big", bufs=1))
        # layer-1 source features, AG-row layout [h0|1|h1|1|h2|1|h3|1|es4]
        hsrc = big.tile([128, T, ROWW], F16)
        ed_b = big.tile([128, H, IC], F16)
        edt_sb = big.tile([H, IC], F16)
        edt_row = big.tile([1, H, IC], F16)
        x1T = big.tile([128, 2, IC], F16)
        x2T = big.tile([128, 2, IC], F16)
        hg_sb = big.tile([128, IC // 128, ROWW], F16)
        den_sb = big.tile([128, H * IC], F32)
        dinv_sb = big.tile([128, H * IC], F32)
        dinv_row = big.tile([1, H * IC], F32)
        dinv_b = big.tile([128, H, IC], F32)
        logit_sb = big.tile([128, IC // 128, 2], F32)

        h65 = hsrc[:, :, 0:H * (C + 1)].rearrange("p t (h x) -> p t h x", h=H)
        nc.vector.memset(h65[:, :, :, C:C + 1], 1.0)
        g65 = hg_sb[:, :, 0:H * (C + 1)].rearrange("p q (h x) -> p q h x",
                                                   h=H)
        nc.vector.memset(g65[:, :, :, C:C + 1], 1.0)

        for layer in (1, 2):
            # ---- prep: edt rows + partition-broadcast to ed_b ----
            with tc.tile_pool(name=f"prep{layer}", bufs=1,
                              space="PSUM") as prep_ps:
                edt_ps = prep_ps.tile([H, IC], F32, tag="edt")
                for lo, sz in ((0, 512), (512, 256)):
                    sl = slice(lo, lo + sz)
                    if layer == 1:
                        nc.tensor.matmul(edt_ps[:, sl], w1d_sb[:, :],
                                         own3_sb[:, sl],
                                         start=True, stop=True)
                    else:
                        for s2 in range(2):
                            nc.tensor.matmul(edt_ps[:, sl],
                                             admw2_sb[:, s2, :],
                                             x1T[:, s2, sl],
                                             start=(s2 == 0), stop=(s2 == 1))
                nc.scalar.copy(edt_sb[:, :], edt_ps[:, :])
            for h in range(H):
                nc.sync.dma_start(out=edt_row[0:1, h, :],
                                  in_=edt_sb[h:h + 1, :])
            for h in range(H):
                nc.gpsimd.partition_broadcast(ed_b[:, h, :],
                                              edt_row[0:1, h, :])

            # ---- slot loop ----
            with tc.tile_pool(name=f"agg_ps{layer}", bufs=1,
                              space="PSUM") as agg_pool:
                agg_ps = agg_pool.tile([128, H, IC], F32, tag="agg",
                                       name=f"agg_{layer}")
                with tc.tile_pool(name=f"jl{layer}", bufs=3) as jl, \
                     tc.tile_pool(name=f"mnp{layer}", bufs=3) as mnp, \
                     tc.tile_pool(name=f"h_ps{layer}", bufs=1,
                                  space="PSUM") as h_psp:
                    for s in range(T):
                        mn = mnp.tile([128, IC], F16, tag="mn",
                                      name=f"mn_{layer}_{s}")
                        if layer == 1:
                            h_ps = h_psp.tile([128, HC], F32, tag="h",
                                              name=f"h_ps_{s}")
                            nc.tensor.matmul(
                                h_ps[:, :],
                                sel5_sb[0:3, s * 128:(s + 1) * 128],
                                w1p_sb[:, :], start=True, stop=True)
                            nc.scalar.copy(
                                h65[:, s, :, 0:C],
                                h_ps[:, :].rearrange("p (h c) -> p h c",
                                                     h=H))
                            # g = R2-d2 (+es cols on 2nd chunk), 2 chunks
                            g_ps = h_psp.tile([128, GA + H], F32, tag="g",
                                              name=f"g_ps_{s}")
                            nc.tensor.matmul(
                                g_ps[:, 0:GA],
                                sel5_sb[:, s * 128:(s + 1) * 128],
                                own5ge_sb[:, 0:GA], start=True, stop=True)
                            nc.vector.tensor_scalar(
                                mn[:, 0:GA], g_ps[:, 0:GA], 0.0, MNEG,
                                OP.is_lt, OP.mult)
                            nc.tensor.matmul(
                                g_ps[:, :],
                                sel5_sb[:, s * 128:(s + 1) * 128],
                                own5ge_sb[:, GA:IC + H],
                                start=True, stop=True)
                            nc.vector.tensor_scalar(
                                mn[:, GA:IC], g_ps[:, 0:IC - GA], 0.0, MNEG,
                                OP.is_lt, OP.mult)
                            nc.vector.tensor_scalar_add(
                                hsrc[:, s, H * (C + 1):ROWW],
                                g_ps[:, IC - GA:IC - GA + H], 0.0)
                            nc.sync.dma_start(out=mn_dram[s, :, :],
                                              in_=mn[:, :])
                            src = hsrc[:, s, :]
                        else:
                            src = jl.tile([128, ROWW], F16, tag="hg",
                                          name=f"hg_{s}")
                            nc.gpsimd.indirect_dma_start(
                                out=src[:, :], out_offset=None,
                                in_=ag_out[:, :],
                                in_offset=bass.IndirectOffsetOnAxis(
                                    ap=agidx_sb[:, s:s + 1], axis=0))
                            nc.sync.dma_start(out=mn[:, :],
                                              in_=mn_dram[s, :, :])

                        # scores: L = leaky02(ed + es + mn); A = exp(L).
                        # STT only has a 1x uop, so: heads 0-1 go through
                        # ACT Prelu (bias=es fuses the add + leaky); heads
                        # 2-3 use one STT then a 4x TS + 2x TT leaky.
                        L4 = jl.tile([128, H, IC], F16, tag="L4",
                                     name=f"L4_{layer}_{s}")
                        u2 = jl.tile([128, 2, IC], F16, tag="u2",
                                     name=f"u2_{layer}_{s}")
                        for h in range(2):
                            nc.vector.tensor_tensor(
                                u2[:, h, :], ed_b[:, h, :], mn[:, :], OP.add)
                            nc.scalar.activation(
                                L4[:, h, :], u2[:, h, :], AF.Prelu,
                                bias=src[:, H * (C + 1) + h:
                                         H * (C + 1) + h + 1],
                                scale=1.0, alpha=0.2)
                        v2 = jl.tile([128, 2, IC], F16, tag="v2",
                                     name=f"v2_{layer}_{s}")
                        for h in range(2, H):
                            nc.vector.scalar_tensor_tensor(
                                v2[:, h - 2, :], ed_b[:, h, :],
                                src[:, H * (C + 1) + h:H * (C + 1) + h + 1],
                                mn[:, :], OP.add, OP.add)
                        t2 = jl.tile([128, 2, IC], F16, tag="t2",
                                     name=f"t2_{layer}_{s}")
                        nc.vector.tensor_scalar_mul(t2[:, :, :], v2[:, :, :],
                                                    0.2)
                        nc.vector.tensor_tensor(L4[:, 2:4, :], v2[:, :, :],
                                                t2[:, :, :], OP.max)
                        A4 = jl.tile([128, H, IC], F16, tag="A4",
                                     name=f"A4_{layer}_{s}")
                        nc.scalar.activation(A4[:, :, :], L4[:, :, :], AF.Exp)
                        if dbg and layer == 1 and s == 0:
                            nc.sync.dma_start(out=dbg_d["dbg_mn0"][:, :],
                                              in_=mn[:, :])
                            nc.sync.dma_start(out=dbg_d["dbg_A0"][:, :, :],
                                              in_=A4[:, :, :])

                        # transposed aggregation: [h|ones] stationary.
                        # 256-col (1KB) chunks keep every matmul output
                        # inside one PSUM bank (head stride is 3KB).
                        # start=True clears has_written for the WHOLE bank,
                        # so only the first-issued region of each bank may
                        # set it (those with q ≡ h mod 2); the bank-mate
                        # region's first write then lands in overwrite mode.
                        for h in range(H):
                            for q in range(3):
                                lo = q * 256
                                nc.tensor.matmul(
                                    agg_ps[0:C + 1, h, lo:lo + 256],
                                    src[:, h * (C + 1):(h + 1) * (C + 1)],
                                    A4[:, h, lo:lo + 256],
                                    start=(s == 0 and (q % 2) == (h % 2)),
                                    stop=(s == T - 1))

                # ---- finalize: x^T = relu(num*dinv + b) ----
                bt_sb = b1t_sb if layer == 1 else b2t_sb
                xT = x1T if layer == 1 else x2T
                with tc.tile_pool(name=f"fin{layer}", bufs=1) as fin:
                    # den: PSUM -> SBUF row -> [128,:] for a lane-parallel
                    # exact reciprocal -> partition-0 row -> broadcast.
                    # Processed per head-pair so the two halves pipeline.
                    xc = fin.tile([C, H, IC], F16, tag="xc")
                    xr = fin.tile([C, H, IC], F16, tag="xr")
                    den_t128 = fin.tile([128, 2, IC // 64], F32,
                                        tag="den128")
                    dinv_t128 = fin.tile([128, 2, IC // 64], F32,
                                         tag="dinv128")
                    HF = 2 * IC  # elements per head-pair
                    for g2 in range(2):
                        hs = slice(2 * g2, 2 * g2 + 2)
                        fs = slice(g2 * HF, (g2 + 1) * HF)
                        nc.scalar.copy(
                            den_sb[C:C + 1, fs],
                            agg_ps[C:C + 1, hs, :].rearrange(
                                "p h d -> p (h d)"))
                        nc.sync.dma_start(out=den_dram[0, fs],
                                          in_=den_sb[C:C + 1, fs])
                        nc.sync.dma_start(
                            out=den_t128[:, g2, :],
                            in_=den_dram[0, fs].rearrange("(a b) -> a b",
                                                          a=128))
                        nc.vector.reciprocal(dinv_t128[:, g2, :],
                                             den_t128[:, g2, :])
                        nc.sync.dma_start(
                            out=den_dram[1, fs].rearrange("(a b) -> a b",
                                                          a=128),
                            in_=dinv_t128[:, g2, :])
                        nc.sync.dma_start(out=dinv_row[0:1, fs],
                                          in_=den_dram[1, fs])
                        nc.gpsimd.partition_broadcast(
                            dinv_b[0:C, hs, :].rearrange(
                                "p h d -> p (h d)"),
                            dinv_row[0:1, fs])
                        nc.vector.tensor_tensor(
                            xc[:, hs, :], agg_ps[0:C, hs, :],
                            dinv_b[0:C, hs, :], OP.mult)
                        for h in range(2 * g2, 2 * g2 + 2):
                            nc.vector.tensor_scalar(
                                xr[:, h, :], xc[:, h, :], bt_sb[:, h:h + 1],
                                0.0, OP.add, OP.max)
                            po = (h % 2) * C
                            nc.sync.dma_start(
                                out=xT[po:po + C, h // 2, :],
                                in_=xr[0:C, h, :])
                    if dbg and layer == 1:
                        nc.sync.dma_start(out=dbg_d["dbg_den"][:, :],
                                          in_=den_sb[C:C + 1, :])
                        nc.sync.dma_start(out=dbg_d["dbg_dinv"][:, :],
                                          in_=dinv_row[0:1, :])

            if dbg and layer == 1:
                nc.sync.dma_start(out=dbg_d["dbg_x1T"][:, :, :],
                                  in_=x1T[:, :, :])
                nc.sync.dma_start(out=dbg_d["dbg_edb"][:, :, :],
                                  in_=ed_b[:, :, :])
                nc.sync.dma_start(out=dbg_d["dbg_hsrc"][:, :, :],
                                  in_=hsrc[:, :, :])
            if layer == 1:
                # ---- h2 rows (+es) for all own nodes; AllGather ----
                with tc.tile_pool(name="h2", bufs=2, space="PSUM") as h2p:
                    for oc in range(IC // 128):
                        h2_ps = h2p.tile([128, HCE], F32, tag="h2",
                                         name=f"h2_{oc}")
                        for s2 in range(2):
                            nc.tensor.matmul(
                                h2_ps[:, :],
                                x1T[:, s2, oc * 128:(oc + 1) * 128],
                                w2p_sb[:, s2, :],
                                start=(s2 == 0), stop=(s2 == 1))
                        nc.scalar.copy(
                            g65[:, oc, :, 0:C],
                            h2_ps[:, 0:HC].rearrange("p (h c) -> p h c",
                                                     h=H))
                        nc.vector.tensor_scalar_add(
                            hg_sb[:, oc, H * (C + 1):ROWW],
                            h2_ps[:, HC:HCE], 0.0)
                nc.sync.dma_start(
                    out=hg_dram.rearrange("(q p) r -> p q r", p=128),
                    in_=hg_sb[:, :, :])
                if fake_ag:
                    for r in range(n_cores):
                        nc.sync.dma_start(
                            out=ag_out[r * IC:(r + 1) * IC, :],
                            in_=hg_dram[:, :])
                else:
                    nc.gpsimd.collective_compute(
                        "AllGather", OP.bypass,
                        replica_groups=[list(range(n_cores))],
                        ins=[hg_dram.opt()],
                        outs=[ag_out.opt()])
            else:
                # ---- fc head ----
                with tc.tile_pool(name="fc", bufs=1, space="PSUM") as fcp:
                    logit_ps = fcp.tile([128, IC // 128, 2], F32, tag="lg")
                    # all 6 chunks share one PSUM bank: single start=True
                    for oc in range(IC // 128):
                        for s2 in range(2):
                            nc.tensor.matmul(
                                logit_ps[:, oc, :],
                                x2T[:, s2, oc * 128:(oc + 1) * 128],
                                fcw_sb[:, s2, :],
                                start=(oc == 0 and s2 == 0), stop=(s2 == 1))
                    for o in range(2):
                        nc.vector.tensor_scalar_add(
                            logit_sb[:, :, o], logit_ps[:, :, o],
                            fcb_sb[:, o:o + 1])
                nc.sync.dma_start(
                    out=out_d.rearrange("(q p) o -> p q o", p=128),
                    in_=logit_sb[:, :, :])

    nc.compile()
    return nc


_BUILD_CACHE = {}


def _get_nc(nslot):
    if nslot not in _BUILD_CACHE:
        _BUILD_CACHE[nslot] = build(nslot)
    return _BUILD_CACHE[nslot]


def _morton(p, bits=10):
    q = np.clip((p * (1 << bits)).astype(np.int64), 0, (1 << bits) - 1)
    code = np.zeros(len(p), np.int64)
    for b in range(bits):
        for dim in range(3):
            code |= ((q[:, dim] >> b) & 1) << (3 * b + dim)
    return code


def _plan(pts):
    """Sort nodes spatially; pick each core's relevant-source node list."""
    order = np.argsort(_morton(pts), kind="stable")
    p_sorted = np.full((KP, 3), PAD_COORD, np.float32)
    p_sorted[:K] = pts[order]

    sq = (p_sorted ** 2).sum(-1, dtype=np.float32)
    G = p_sorted @ p_sorted.T
    d2 = sq[None, :] + sq[:, None] - 2.0 * G
    near = d2 < (R2 + MASK_EPS)          # [src, dst], conservative superset

    srcs_list = []
    for c in range(N_CORES):
        srcs = np.flatnonzero(near[:, c * IC:(c + 1) * IC].any(axis=1))
        srcs_list.append(srcs)
    T = max(-(-len(s) // 128) for s in srcs_list)
    srcs_list = [np.concatenate(
        [s, np.full(T * 128 - len(s), PAD_NODE, s.dtype)])
        for s in srcs_list]
    return order, p_sorted, srcs_list, T


def _blockdiag(a):  # [H, C] -> [HC, H] fp32
    m = np.zeros((HC, H), dtype=np.float32)
    for h in range(H):
        m[h * C:(h + 1) * C, h] = np.asarray(a, np.float32)[h]
    return m


def _prep_inputs(pos, pos_non_manifold, W1, a_src1, a_dst1, b1,
                 W2, a_src2, a_dst2, b2, fc_w, fc_b):
    f16 = np.float16
    pts = np.concatenate([np.asarray(pos, np.float32),
                          np.asarray(pos_non_manifold, np.float32)],
                         axis=2)[0].T  # [K, 3]
    order, p_sorted, srcs_list, T = _plan(pts)
    sq_sorted = (p_sorted ** 2).sum(-1, dtype=np.float32)

    W1f = np.asarray(W1, np.float32)
    W2f = np.asarray(W2, np.float32)
    w1s = W1f @ _blockdiag(a_src1)            # [3, H]
    w2p = np.concatenate([W2f, W2f @ _blockdiag(a_src2)], axis=1)

    shared = {
        "w1p": np.ascontiguousarray(W1f),
        "w1d": np.ascontiguousarray(W1f @ _blockdiag(a_dst1)),
        "w2p": np.ascontiguousarray(w2p.astype(f16)),
        "admw2": np.ascontiguousarray(
            (W2f @ _blockdiag(a_dst2)).astype(f16)),
        "b1t": np.ascontiguousarray(
            np.asarray(b1, np.float32).reshape(H, C).T),
        "b2t": np.ascontiguousarray(
            np.asarray(b2, np.float32).reshape(H, C).T),
        "fcw": np.ascontiguousarray(np.asarray(fc_w, np.float32).astype(f16)),
        "fcb": np.ascontiguousarray(np.broadcast_to(
            np.asarray(fc_b, np.float32).reshape(1, 2), (128, 2))),
    }
    in_maps = []
    for c in range(N_CORES):
        srcs = srcs_list[c]
        psel = p_sorted[srcs]                     # [T*128, 3]
        pown = p_sorted[c * IC:(c + 1) * IC]
        sel5 = np.concatenate(
            [psel.T, sq_sorted[srcs][None, :],
             np.ones((1, len(srcs)), np.float32)], axis=0)
        own5 = np.concatenate(
            [2.0 * pown.T, -np.ones((1, IC), np.float32),
             (R2 - sq_sorted[c * IC:(c + 1) * IC])[None, :]], axis=0)
        es_cols = np.concatenate(
            [w1s, np.zeros((2, H), np.float32)], axis=0)  # [5, H]
        m = dict(shared)
        m["sel5"] = np.ascontiguousarray(sel5)
        m["own5ge"] = np.ascontiguousarray(
            np.concatenate([own5, es_cols], axis=1))
        m["own3"] = np.ascontiguousarray(pown.T)
        m["agidx"] = np.ascontiguousarray(
            srcs.reshape(T, 128).T.astype(np.int32))
        in_maps.append(m)
    return in_maps, order, T


def kernel(pos, pos_non_manifold, W1, a_src1, a_dst1, b1,
           W2, a_src2, a_dst2, b2, fc_w, fc_b, _trace=False):
    in_maps, order, T = _prep_inputs(
        pos, pos_non_manifold, W1, a_src1, a_dst1, b1,
        W2, a_src2, a_dst2, b2, fc_w, fc_b)
    nc = _get_nc(T)
    res = run_bass_kernel_spmd(nc, in_maps, core_ids=list(range(N_CORES)),
                               trace=_trace)
    kernel.last_results = res
    x2s = np.concatenate([res.results[c]["out"] for c in range(N_CORES)],
                         axis=0)  # [KP, 2] in sorted order
    x2 = np.empty((K, 2), np.float32)
    x2[order] = x2s[:K]
    logits = np.ascontiguousarray(x2[M:K]).reshape(1, 2, 3000)
    return logits.astype(np.float32)


# revision 31
# speedup vs baseline: 2.8668x; 1.0106x over previous
"""Trainium2 Bass kernel for a 2-layer GAT occupancy predictor (B=1).

Reference math:
  pts = concat(pos, pos_non_manifold) -> [K=6000, 3]
  mask[i,j] = ||pts_i - pts_j||^2 < 0.05^2          (dense radius graph)
  layer l:  h = x @ Wl                              [K, 4*64]
            e[i,j,h] = leaky02(ed[i,h] + es[j,h])   es/ed = <h, a_src/dst>
            alpha = softmax_j(e masked)
            x' = relu(alpha @ h + b)
  logits = (x2 @ fc_w + fc_b)[M:] reshaped to [1, 2, 3000]

Distribution (8 NeuronCores): nodes are Morton-sorted; core c owns the 768
destinations [768c, 768(c+1)) of the padded 6144-node graph.  Each core's
sources are CUSTOM-PACKED: only the ~900 nodes within radius of its block,
gathered into T=ceil(max_unique/128) tiles of 128 (padded with node 6143),
instead of whole global Morton tiles.  This cuts per-core source tiles from
~28 to ~8 and makes dense-768-dst processing cheap enough to skip chunking.

Everything 16-bit on the hot path (fp16), f32 accumulation in PSUM:
  per slot s (128 sources x 768 dsts x 4 heads):
    PE   : layer1 h = p @ W1 [128,256]; g = (R2-d2 | es-cols) via one K=5
           matmul; transposed aggregation x^T[c,dst] += A.h with [h|ones]
           stationary (denominator rides as the 65th weight column).
    gpsimd: mask mn = (g<0)*-60000 (psum->fp16), layer-2 row gathers
           (indirect DMA), ed/deninv partition broadcasts.
    DVE  : per head ONE fused v_h = (ed_h + es_h) + mn  (scalar_tensor_tensor)
           then ONE batched leaky L = max(0.2v, v) over all heads.
    ACT  : one exp over [128, 4*768], h copies.
  Between layers: x1^T assembled by 4 partition-moving DMAs; h2 = x1 @ W2
  (+es ride-along) computed per-owner, AllGathered as fp16 node-major rows
  [h0|1|h1|1|h2|1|h3|1|es4]; layer 2 fetches each slot's rows with one
  indirect DMA.  Masks bounce through DRAM between layers.
"""

import sys

sys.path.insert(0, "/opt/trn_rl_repo")

from contextlib import ExitStack

import ml_dtypes
import numpy as np

import concourse.bacc as bacc
import concourse.bass as bass
import concourse.mybir as mybir
import concourse.tile as tile
from concourse.bass_utils import run_bass_kernel_spmd

F32 = mybir.dt.float32
F16 = mybir.dt.float16
I32 = mybir.dt.int32
AF = mybir.ActivationFunctionType
OP = mybir.AluOpType
AX = mybir.AxisListType

N_CORES = 8
N = 3000
M = 3000
K = N + M          # real nodes
KP = 6144          # padded nodes
IC = KP // N_CORES # 768 destinations per core
H = 4              # heads
C = 64             # channels per head
HC = H * C         # 256
HCE = HC + H       # 260: h columns + es columns (layer-2 ride-along)
ROWW = H * (C + 1) + H  # 264: AG row [h0|1|h1|1|h2|1|h3|1|es4]
R2 = float(np.float32(0.05) * np.float32(0.05))
PAD_COORD = -1.0
PAD_NODE = KP - 1
MASK_EPS = 1e-5    # host activity-test margin (superset of device mask)
MNEG = -60000.0    # masked-score offset; *0.2 then exp -> 0 in fp16
GA = 384           # d2/mask column chunk (PSUM bank budget)


def build(nslot, n_cores=N_CORES, fake_ag=False, dbg=False):
    nc = bacc.Bacc("TRN2", target_bir_lowering=False, debug=False,
                   num_devices=n_cores)
    T = nslot
    dbg_d = {}
    if dbg:
        for nm, shp, dt in (("dbg_den", [1, H * IC], F32),
                            ("dbg_dinv", [1, H * IC], F32),
                            ("dbg_x1T", [128, 2, IC], F16),
                            ("dbg_edb", [128, H, IC], F16),
                            ("dbg_mn0", [128, IC], F16),
                            ("dbg_A0", [128, H, IC], F16),
                            ("dbg_hsrc", [128, nslot, ROWW], F16)):
            dbg_d[nm] = nc.dram_tensor(nm, shp, dt, kind="ExternalOutput")

    # ---- kernel I/O (identical program on every core) ----
    sel5_d = nc.dram_tensor("sel5", [5, T * 128], F32, kind="ExternalInput")
    # own5ge: cols 0:768 = [2p; -1; R2-sq] (g = R2-d2), cols 768:772 = es1
    own5ge_d = nc.dram_tensor("own5ge", [5, IC + H], F32,
                              kind="ExternalInput")
    own3_d = nc.dram_tensor("own3", [3, IC], F32, kind="ExternalInput")
    agidx_d = nc.dram_tensor("agidx", [128, T], I32, kind="ExternalInput")
    w1p_d = nc.dram_tensor("w1p", [3, HC], F32, kind="ExternalInput")
    w1d_d = nc.dram_tensor("w1d", [3, H], F32, kind="ExternalInput")
    w2p_d = nc.dram_tensor("w2p", [HC, HCE], F16, kind="ExternalInput")
    admw2_d = nc.dram_tensor("admw2", [HC, H], F16, kind="ExternalInput")
    b1t_d = nc.dram_tensor("b1t", [C, H], F32, kind="ExternalInput")
    b2t_d = nc.dram_tensor("b2t", [C, H], F32, kind="ExternalInput")
    fcw_d = nc.dram_tensor("fcw", [HC, 2], F16, kind="ExternalInput")
    fcb_d = nc.dram_tensor("fcb", [128, 2], F32, kind="ExternalInput")

    out_d = nc.dram_tensor("out", [IC, 2], F32, kind="ExternalOutput")

    with tile.TileContext(nc) as tc, ExitStack() as st:
        dram = st.enter_context(tc.tile_pool(name="dram", bufs=1,
                                             space="DRAM"))
        mn_dram = dram.tile([T, 128, IC], F16)
        den_dram = dram.tile([2, H * IC], F32)
        hg_dram = dram.tile([IC, ROWW], F16)
        ag_out = dram.tile([KP, ROWW], F16,
                           addr_space=("Local" if fake_ag else "Shared"))

        const = st.enter_context(tc.tile_pool(name="const", bufs=1))
        sel5_sb = const.tile([5, T * 128], F32)
        own5ge_sb = const.tile([5, IC + H], F32)
        own3_sb = const.tile([3, IC], F32)
        agidx_sb = const.tile([128, T], I32)
        w1p_sb = const.tile([3, HC], F32)
        w1d_sb = const.tile([3, H], F32)
        w2p_sb = const.tile([128, 2, HCE], F16)
        admw2_sb = const.tile([128, 2, H], F16)
        b1t_sb = const.tile([C, H], F32)
        b2t_sb = const.tile([C, H], F32)
        fcw_sb = const.tile([128, 2, 2], F16)
        fcb_sb = const.tile([128, 2], F32)

        nc.sync.dma_start(out=sel5_sb[:, :], in_=sel5_d[:, :])
        nc.sync.dma_start(out=own5ge_sb[:, :], in_=own5ge_d[:, :])
        nc.sync.dma_start(out=own3_sb[:, :], in_=own3_d[:, :])
        nc.sync.dma_start(out=agidx_sb[:, :], in_=agidx_d[:, :])
        nc.sync.dma_start(out=w1p_sb[:, :], in_=w1p_d[:, :])
        nc.sync.dma_start(out=w1d_sb[:, :], in_=w1d_d[:, :])
        nc.sync.dma_start(out=w2p_sb[:, :, :],
                          in_=w2p_d.rearrange("(s p) c -> p s c", p=128))
        nc.sync.dma_start(out=admw2_sb[:, :, :],
                          in_=admw2_d.rearrange("(s p) h -> p s h", p=128))
        nc.sync.dma_start(out=b1t_sb[:, :], in_=b1t_d[:, :])
        nc.sync.dma_start(out=b2t_sb[:, :], in_=b2t_d[:, :])
        nc.sync.dma_start(out=fcw_sb[:, :, :],
                          in_=fcw_d.rearrange("(s p) o -> p s o", p=128))
        nc.sync.dma_start(out=fcb_sb[:, :], in_=fcb_d[:, :])

        big = st.enter_context(tc.tile_pool(name="big", bufs=1))
        # layer-1 source features, AG-row layout [h0|1|h1|1|h2|1|h3|1|es4]
        hsrc = big.tile([128, T, ROWW], F16)
        es4f = big.tile([128, T, H], F32)
        ed_b = big.tile([128, H, IC], F16)
        edt_sb = big.tile([H, IC], F16)
        edt_row = big.tile([1, H, IC], F16)
        x1T = big.tile([128, 2, IC], F16)
        x2T = big.tile([128, 2, IC], F16)
        hg_sb = big.tile([128, IC // 128, ROWW], F16)
        den_sb = big.tile([128, H * IC], F32)
        dinv_sb = big.tile([128, H * IC], F32)
        dinv_row = big.tile([1, H * IC], F32)
        dinv_b = big.tile([128, H, IC], F32)
        logit_sb = big.tile([128, IC // 128, 2], F32)

        h65 = hsrc[:, :, 0:H * (C + 1)].rearrange("p t (h x) -> p t h x", h=H)
        nc.vector.memset(h65[:, :, :, C:C + 1], 1.0)
        g65 = hg_sb[:, :, 0:H * (C + 1)].rearrange("p q (h x) -> p q h x",
                                                   h=H)
        nc.vector.memset(g65[:, :, :, C:C + 1], 1.0)

        for layer in (1, 2):
            # ---- prep: edt rows + partition-broadcast to ed_b ----
            with tc.tile_pool(name=f"prep{layer}", bufs=1,
                              space="PSUM") as prep_ps:
                edt_ps = prep_ps.tile([H, IC], F32, tag="edt")
                for lo, sz in ((0, 512), (512, 256)):
                    sl = slice(lo, lo + sz)
                    if layer == 1:
                        nc.tensor.matmul(edt_ps[:, sl], w1d_sb[:, :],
                                         own3_sb[:, sl],
                                         start=True, stop=True)
                    else:
                        for s2 in range(2):
                            nc.tensor.matmul(edt_ps[:, sl],
                                             admw2_sb[:, s2, :],
                                             x1T[:, s2, sl],
                                             start=(s2 == 0), stop=(s2 == 1))
                nc.scalar.copy(edt_sb[:, :], edt_ps[:, :])
            for h in range(H):
                nc.sync.dma_start(out=edt_row[0:1, h, :],
                                  in_=edt_sb[h:h + 1, :])
            for h in range(H):
                nc.gpsimd.partition_broadcast(ed_b[:, h, :],
                                              edt_row[0:1, h, :])

            # ---- slot loop ----
            with tc.tile_pool(name=f"agg_ps{layer}", bufs=1,
                              space="PSUM") as agg_pool:
                agg_ps = agg_pool.tile([128, H, IC], F32, tag="agg",
                                       name=f"agg_{layer}")
                with tc.tile_pool(name=f"jl{layer}", bufs=3) as jl, \
                     tc.tile_pool(name=f"mnp{layer}", bufs=3) as mnp, \
                     tc.tile_pool(name=f"h_ps{layer}", bufs=1,
                                  space="PSUM") as h_psp:
                    for s in range(T):
                        mn = mnp.tile([128, IC], F16, tag="mn",
                                      name=f"mn_{layer}_{s}")
                        if layer == 1:
                            h_ps = h_psp.tile([128, HC], F32, tag="h",
                                              name=f"h_ps_{s}")
                            nc.tensor.matmul(
                                h_ps[:, :],
                                sel5_sb[0:3, s * 128:(s + 1) * 128],
                                w1p_sb[:, :], start=True, stop=True)
                            nc.scalar.copy(
                                h65[:, s, :, 0:C],
                                h_ps[:, :].rearrange("p (h c) -> p h c",
                                                     h=H))
                            # g = R2-d2 (+es cols on 2nd chunk), 2 chunks
                            g_ps = h_psp.tile([128, GA + H], F32, tag="g",
                                              name=f"g_ps_{s}")
                            nc.tensor.matmul(
                                g_ps[:, 0:GA],
                                sel5_sb[:, s * 128:(s + 1) * 128],
                                own5ge_sb[:, 0:GA], start=True, stop=True)
                            nc.vector.tensor_scalar(
                                mn[:, 0:GA], g_ps[:, 0:GA], 0.0, MNEG,
                                OP.is_lt, OP.mult)
                            nc.tensor.matmul(
                                g_ps[:, :],
                                sel5_sb[:, s * 128:(s + 1) * 128],
                                own5ge_sb[:, GA:IC + H],
                                start=True, stop=True)
                            nc.vector.tensor_scalar(
                                mn[:, GA:IC], g_ps[:, 0:IC - GA], 0.0, MNEG,
                                OP.is_lt, OP.mult)
                            nc.vector.tensor_scalar_add(
                                es4f[:, s, :],
                                g_ps[:, IC - GA:IC - GA + H], 0.0)
                            nc.sync.dma_start(out=mn_dram[s, :, :],
                                              in_=mn[:, :])
                            src = hsrc[:, s, :]
                            es_ap = es4f[:, s, :]
                        else:
                            src = jl.tile([128, ROWW], F16, tag="hg",
                                          name=f"hg_{s}")
                            nc.gpsimd.indirect_dma_start(
                                out=src[:, :], out_offset=None,
                                in_=ag_out[:, :],
                                in_offset=bass.IndirectOffsetOnAxis(
                                    ap=agidx_sb[:, s:s + 1], axis=0))
                            nc.sync.dma_start(out=mn[:, :],
                                              in_=mn_dram[s, :, :])
                            esg = jl.tile([128, H], F32, tag="esg",
                                          name=f"esg_{s}")
                            nc.vector.tensor_scalar_add(
                                esg[:, :],
                                src[:, H * (C + 1):ROWW], 0.0)
                            es_ap = esg[:, :]

                        # scores: L = leaky02(ed + es + mn); A = exp(L).
                        # u4 = ed + mn in ONE 2x TT via a stride-0 head
                        # broadcast of mn.  Then heads 0-1 get es+leaky via
                        # ACT Prelu (bias=es); heads 2-3 via 4x TS es-adds
                        # and a TS/TT leaky (STT only has a 1x uop).
                        L4 = jl.tile([128, H, IC], F16, tag="L4",
                                     name=f"L4_{layer}_{s}")
                        u4 = jl.tile([128, H, IC], F16, tag="u4",
                                     name=f"u4_{layer}_{s}")
                        ub, mb = bass.broadcast_tensor_aps(
                            ed_b[:, :, :],
                            mn[:, :].rearrange("p (o d) -> p o d", o=1))
                        nc.vector.tensor_tensor(u4[:, :, :], ub, mb, OP.add)
                        for h in range(2):
                            nc.scalar.activation(
                                L4[:, h, :], u4[:, h, :], AF.Prelu,
                                bias=es_ap[:, h:h + 1],
                                scale=1.0, alpha=0.2)
                        v2 = jl.tile([128, 2, IC], F16, tag="v2",
                                     name=f"v2_{layer}_{s}")
                        for h in range(2, H):
                            nc.vector.tensor_scalar_add(
                                v2[:, h - 2, :], u4[:, h, :],
                                es_ap[:, h:h + 1])
                        t2 = jl.tile([128, 2, IC], F16, tag="t2",
                                     name=f"t2_{layer}_{s}")
                        nc.vector.tensor_scalar_mul(t2[:, :, :], v2[:, :, :],
                                                    0.2)
                        nc.vector.tensor_tensor(L4[:, 2:4, :], v2[:, :, :],
                                                t2[:, :, :], OP.max)
                        A4 = jl.tile([128, H, IC], F16, tag="A4",
                                     name=f"A4_{layer}_{s}")
                        nc.scalar.activation(A4[:, :, :], L4[:, :, :], AF.Exp)
                        if dbg and layer == 1 and s == 0:
                            nc.sync.dma_start(out=dbg_d["dbg_mn0"][:, :],
                                              in_=mn[:, :])
                            nc.sync.dma_start(out=dbg_d["dbg_A0"][:, :, :],
                                              in_=A4[:, :, :])

                        # transposed aggregation: [h|ones] stationary.
                        # 256-col (1KB) chunks keep every matmul output
                        # inside one PSUM bank (head stride is 3KB).
                        # start=True clears has_written for the WHOLE bank,
                        # so only the first-issued region of each bank may
                        # set it (those with q ≡ h mod 2); the bank-mate
                        # region's first write then lands in overwrite mode.
                        for h in range(H):
                            for q in range(3):
                                lo = q * 256
                                nc.tensor.matmul(
                                    agg_ps[0:C + 1, h, lo:lo + 256],
                                    src[:, h * (C + 1):(h + 1) * (C + 1)],
                                    A4[:, h, lo:lo + 256],
                                    start=(s == 0 and (q % 2) == (h % 2)),
                                    stop=(s == T - 1))

                # ---- finalize: x^T = relu(num*dinv + b) ----
                bt_sb = b1t_sb if layer == 1 else b2t_sb
                xT = x1T if layer == 1 else x2T
                with tc.tile_pool(name=f"fin{layer}", bufs=1) as fin:
                    # den: PSUM -> SBUF row -> [128,:] for a lane-parallel
                    # exact reciprocal -> partition-0 row -> broadcast.
                    # Processed per head-pair so the two halves pipeline.
                    xc = fin.tile([C, H, IC], F16, tag="xc")
                    xr = fin.tile([C, H, IC], F16, tag="xr")
                    den_t128 = fin.tile([128, 2, IC // 64], F32,
                                        tag="den128")
                    dinv_t128 = fin.tile([128, 2, IC // 64], F32,
                                         tag="dinv128")
                    HF = 2 * IC  # elements per head-pair
                    for g2 in range(2):
                        hs = slice(2 * g2, 2 * g2 + 2)
                        fs = slice(g2 * HF, (g2 + 1) * HF)
                        nc.scalar.copy(
                            den_sb[C:C + 1, fs],
                            agg_ps[C:C + 1, hs, :].rearrange(
                                "p h d -> p (h d)"))
                        nc.sync.dma_start(out=den_dram[0, fs],
                                          in_=den_sb[C:C + 1, fs])
                        nc.sync.dma_start(
                            out=den_t128[:, g2, :],
                            in_=den_dram[0, fs].rearrange("(a b) -> a b",
                                                          a=128))
                        nc.vector.reciprocal(dinv_t128[:, g2, :],
                                             den_t128[:, g2, :])
                        nc.sync.dma_start(
                            out=den_dram[1, fs].rearrange("(a b) -> a b",
                                                          a=128),
                            in_=dinv_t128[:, g2, :])
                        nc.sync.dma_start(out=dinv_row[0:1, fs],
                                          in_=den_dram[1, fs])
                        nc.gpsimd.partition_broadcast(
                            dinv_b[0:C, hs, :].rearrange(
                                "p h d -> p (h d)"),
                            dinv_row[0:1, fs])
                        nc.vector.tensor_tensor(
                            xc[:, hs, :], agg_ps[0:C, hs, :],
                            dinv_b[0:C, hs, :], OP.mult)
                        for h in range(2 * g2, 2 * g2 + 2):
                            nc.vector.tensor_scalar(
                                xr[:, h, :], xc[:, h, :], bt_sb[:, h:h + 1],
                                0.0, OP.add, OP.max)
                            po = (h % 2) * C
                            nc.sync.dma_start(
                                out=xT[po:po + C, h // 2, :],
                                in_=xr[0:C, h, :])
                    if dbg and layer == 1:
                        nc.sync.dma_start(out=dbg_d["dbg_den"][:, :],
                                          in_=den_sb[C:C + 1, :])
                        nc.sync.dma_start(out=dbg_d["dbg_dinv"][:, :],
                                          in_=dinv_row[0:1, :])

            if dbg and layer == 1:
                nc.sync.dma_start(out=dbg_d["dbg_x1T"][:, :, :],
                                  in_=x1T[:, :, :])
                nc.sync.dma_start(out=dbg_d["dbg_edb"][:, :, :],
                                  in_=ed_b[:, :, :])
                nc.sync.dma_start(out=dbg_d["dbg_hsrc"][:, :, :],
                                  in_=hsrc[:, :, :])
            if layer == 1:
                # ---- h2 rows (+es) for all own nodes; AllGather ----
                with tc.tile_pool(name="h2", bufs=2, space="PSUM") as h2p:
                    for oc in range(IC // 128):
                        h2_ps = h2p.tile([128, HCE], F32, tag="h2",
                                         name=f"h2_{oc}")
                        for s2 in range(2):
                            nc.tensor.matmul(
                                h2_ps[:, :],
                                x1T[:, s2, oc * 128:(oc + 1) * 128],
                                w2p_sb[:, s2, :],
                                start=(s2 == 0), stop=(s2 == 1))
                        nc.scalar.copy(
                            g65[:, oc, :, 0:C],
                            h2_ps[:, 0:HC].rearrange("p (h c) -> p h c",
                                                     h=H))
                        nc.vector.tensor_scalar_add(
                            hg_sb[:, oc, H * (C + 1):ROWW],
                            h2_ps[:, HC:HCE], 0.0)
                nc.sync.dma_start(
                    out=hg_dram.rearrange("(q p) r -> p q r", p=128),
                    in_=hg_sb[:, :, :])
                if fake_ag:
                    for r in range(n_cores):
                        nc.sync.dma_start(
                            out=ag_out[r * IC:(r + 1) * IC, :],
                            in_=hg_dram[:, :])
                else:
                    nc.gpsimd.collective_compute(
                        "AllGather", OP.bypass,
                        replica_groups=[list(range(n_cores))],
                        ins=[hg_dram.opt()],
                        outs=[ag_out.opt()])
            else:
                # ---- fc head ----
                with tc.tile_pool(name="fc", bufs=1, space="PSUM") as fcp:
                    logit_ps = fcp.tile([128, IC // 128, 2], F32, tag="lg")
                    # all 6 chunks share one PSUM bank: single start=True
                    for oc in range(IC // 128):
                        for s2 in range(2):
                            nc.tensor.matmul(
                                logit_ps[:, oc, :],
                                x2T[:, s2, oc * 128:(oc + 1) * 128],
                                fcw_sb[:, s2, :],
                                start=(oc == 0 and s2 == 0), stop=(s2 == 1))
                    for o in range(2):
                        nc.vector.tensor_scalar_add(
                            logit_sb[:, :, o], logit_ps[:, :, o],
                            fcb_sb[:, o:o + 1])
                nc.sync.dma_start(
                    out=out_d.rearrange("(q p) o -> p q o", p=128),
                    in_=logit_sb[:, :, :])

    nc.compile()
    return nc


_BUILD_CACHE = {}


def _get_nc(nslot):
    if nslot not in _BUILD_CACHE:
        _BUILD_CACHE[nslot] = build(nslot)
    return _BUILD_CACHE[nslot]


def _morton(p, bits=10):
    q = np.clip((p * (1 << bits)).astype(np.int64), 0, (1 << bits) - 1)
    code = np.zeros(len(p), np.int64)
    for b in range(bits):
        for dim in range(3):
            code |= ((q[:, dim] >> b) & 1) << (3 * b + dim)
    return code


def _plan(pts):
    """Sort nodes spatially; pick each core's relevant-source node list."""
    order = np.argsort(_morton(pts), kind="stable")
    p_sorted = np.full((KP, 3), PAD_COORD, np.float32)
    p_sorted[:K] = pts[order]

    sq = (p_sorted ** 2).sum(-1, dtype=np.float32)
    G = p_sorted @ p_sorted.T
    d2 = sq[None, :] + sq[:, None] - 2.0 * G
    near = d2 < (R2 + MASK_EPS)          # [src, dst], conservative superset

    srcs_list = []
    for c in range(N_CORES):
        srcs = np.flatnonzero(near[:, c * IC:(c + 1) * IC].any(axis=1))
        srcs_list.append(srcs)
    T = max(-(-len(s) // 128) for s in srcs_list)
    srcs_list = [np.concatenate(
        [s, np.full(T * 128 - len(s), PAD_NODE, s.dtype)])
        for s in srcs_list]
    return order, p_sorted, srcs_list, T


def _blockdiag(a):  # [H, C] -> [HC, H] fp32
    m = np.zeros((HC, H), dtype=np.float32)
    for h in range(H):
        m[h * C:(h + 1) * C, h] = np.asarray(a, np.float32)[h]
    return m


def _prep_inputs(pos, pos_non_manifold, W1, a_src1, a_dst1, b1,
                 W2, a_src2, a_dst2, b2, fc_w, fc_b):
    f16 = np.float16
    pts = np.concatenate([np.asarray(pos, np.float32),
                          np.asarray(pos_non_manifold, np.float32)],
                         axis=2)[0].T  # [K, 3]
    order, p_sorted, srcs_list, T = _plan(pts)
    sq_sorted = (p_sorted ** 2).sum(-1, dtype=np.float32)

    W1f = np.asarray(W1, np.float32)
    W2f = np.asarray(W2, np.float32)
    w1s = W1f @ _blockdiag(a_src1)            # [3, H]
    w2p = np.concatenate([W2f, W2f @ _blockdiag(a_src2)], axis=1)

    shared = {
        "w1p": np.ascontiguousarray(W1f),
        "w1d": np.ascontiguousarray(W1f @ _blockdiag(a_dst1)),
        "w2p": np.ascontiguousarray(w2p.astype(f16)),
        "admw2": np.ascontiguousarray(
            (W2f @ _blockdiag(a_dst2)).astype(f16)),
        "b1t": np.ascontiguousarray(
            np.asarray(b1, np.float32).reshape(H, C).T),
        "b2t": np.ascontiguousarray(
            np.asarray(b2, np.float32).reshape(H, C).T),
        "fcw": np.ascontiguousarray(np.asarray(fc_w, np.float32).astype(f16)),
        "fcb": np.ascontiguousarray(np.broadcast_to(
            np.asarray(fc_b, np.float32).reshape(1, 2), (128, 2))),
    }
    in_maps = []
    for c in range(N_CORES):
        srcs = srcs_list[c]
        psel = p_sorted[srcs]                     # [T*128, 3]
        pown = p_sorted[c * IC:(c + 1) * IC]
        sel5 = np.concatenate(
            [psel.T, sq_sorted[srcs][None, :],
             np.ones((1, len(srcs)), np.float32)], axis=0)
        own5 = np.concatenate(
            [2.0 * pown.T, -np.ones((1, IC), np.float32),
             (R2 - sq_sorted[c * IC:(c + 1) * IC])[None, :]], axis=0)
        es_cols = np.concatenate(
            [w1s, np.zeros((2, H), np.float32)], axis=0)  # [5, H]
        m = dict(shared)
        m["sel5"] = np.ascontiguousarray(sel5)
        m["own5ge"] = np.ascontiguousarray(
            np.concatenate([own5, es_cols], axis=1))
        m["own3"] = np.ascontiguousarray(pown.T)
        m["agidx"] = np.ascontiguousarray(
            srcs.reshape(T, 128).T.astype(np.int32))
        in_maps.append(m)
    return in_maps, order, T


def kernel(pos, pos_non_manifold, W1, a_src1, a_dst1, b1,
           W2, a_src2, a_dst2, b2, fc_w, fc_b, _trace=False):
    in_maps, order, T = _prep_inputs(
        pos, pos_non_manifold, W1, a_src1, a_dst1, b1,
        W2, a_src2, a_dst2, b2, fc_w, fc_b)
    nc = _get_nc(T)
    res = run_bass_kernel_spmd(nc, in_maps, core_ids=list(range(N_CORES)),
                               trace=_trace)
    kernel.last_results = res
    x2s = np.concatenate([res.results[c]["out"] for c in range(N_CORES)],
                         axis=0)  # [KP, 2] in sorted order
    x2 = np.empty((K, 2), np.float32)
    x2[order] = x2s[:K]
    logits = np.ascontiguousarray(x2[M:K]).reshape(1, 2, 3000)
    return logits.astype(np.float32)
